# revision 23
# baseline (speedup 1.0000x reference)
"""Trainium2 Bass kernel for nn_AllInOneLayer (GNN message-passing layer).

Sharding: data-parallel over graphs. 8 cores, B/8 graphs each; nodes/edges
of a core's graphs are contiguous (batch is sorted). Edges are assigned to
the core that owns their DESTINATION node (row = edge_index[0]); within a
core they are grouped per 128-node tile with a uniform padded slot count S,
so the segment softmax and scatter-adds become core-local PE matmuls with
host-built one-hot matrices. All float math runs on the device; the host
only does index manipulation / gathers / layout packing.
"""

import contextlib
import math

import numpy as np

import concourse.bass as bass
import concourse.tile as tile
from concourse import mybir
from concourse.bass_utils import run_bass_kernel_spmd
from concourse.masks import make_identity

AF = mybir.ActivationFunctionType
OP = mybir.AluOpType
F32 = mybir.dt.float32
F32R = mybir.dt.float32r

C = 8           # cores
DS, DV, DU, HH = 128, 15, 128, 4
HID_A = DS + DV           # 143, attn hidden per head
MA = HH * HID_A           # 572, stacked attn hidden
USE_F32R = True           # relaxed-precision matmuls (4x faster, ~1e-4 rel err)
FR = F32R if USE_F32R else F32   # dtype for every matmul-feeding tensor

_BUILD_CACHE = {}


def _r(ap):
    """Matmul operands already carry FR dtype; kept for call-site clarity."""
    return ap


def _split_waits(nc, limit=1):
    """This toolchain's walrus accepts at most one sync-wait per instruction;
    move excess waits onto preceding same-engine NoOps."""
    for f in nc.m.functions:
        for bb in f.blocks:
            new = []
            for inst in bb.instructions:
                si = inst.sync_info
                waits = list(si.on_wait) if si else []
                if len(waits) > limit:
                    chunks = [waits[i:i + limit] for i in range(0, len(waits), limit)]
                    for ch in chunks[:-1]:
                        nop = mybir.InstNoOp(
                            name=nc.get_next_instruction_name(), ins=[], outs=[])
                        nop.engine = inst.engine
                        nop.sync_info = mybir.SyncInfo(on_wait=list(ch), on_update=[])
                        new.append(nop)
                    inst.sync_info = mybir.SyncInfo(
                        on_wait=list(chunks[-1]), on_update=list(si.on_update))
                new.append(inst)
            bb.instructions = new


# weight/bias dram shapes (lhsT layout [K_in, M_out]; biases [p, cols])
_WSHAPES = [
    ("wn1", [HID_A, 2 * DU]), ("wn2", [2 * DU, DU]),
    ("we1", [HID_A, 2 * DU]), ("we2", [2 * DU, DU]),
    ("wm1", [3 * DU, 2 * DU]), ("wf", [6 * DU, 2 * DU]),
    ("wm2", [2 * DU, DS]),
    ("ws1", [DS, DS // 2]), ("ws2", [DS // 2, 1]),
    ("wa1", [3 * DS, MA]), ("wa1v", [80, MA]), ("wa2", [MA, HH]),
    ("wnev", [80, 2 * DU]),
    ("wsd1", [3 * DS, DS]), ("wsd2", [DS, DS]),
    ("wvd1", [3 * DV, DV]), ("wvd2", [DV, DV]),
    ("wg1", [DS + 3, DS]), ("wg2", [DS, 1]),
    ("wsc1", [3 * DS, DS]), ("wsc2", [DS, DS]),
    ("wvc1", [3 * DV, DV]), ("wvc2", [DV, DV]),
    ("bn1", [128, 2]), ("be1", [128, 2]),
    ("bm1", [128, 2]), ("bm2", [128, 1]),
    ("bs1", [64, 1]), ("bs2", [1, 1]),
    ("ba1", [128, 5]), ("ba2", [4, 1]),
    ("bsd1", [128, 1]), ("bsd2", [128, 1]),
    ("bvd1", [15, 1]), ("bvd2", [15, 1]),
    ("bg1", [128, 1]), ("bg2", [1, 1]),
    ("bsc1", [128, 1]), ("bsc2", [128, 1]),
    ("bvc1", [15, 1]), ("bvc2", [15, 1]),
    ("gnw", [128, 1]), ("gnb", [128, 1]), ("gnms", [128, 1]),
    ("lnw", [128, 1]), ("lnb", [128, 1]),
]
# loaded whole into one sbuf tile (partition dim <= 128)
_WSMALL = {"ws1", "ws2", "wsd2", "wvd2", "wg2", "wsc2",
           "wvc2", "bn1", "be1", "bm1", "bm2", "bs1", "bs2", "ba1", "ba2",
           "bsd1", "bsd2", "bvd1", "bvd2", "bg1", "bg2", "bsc1", "bsc2",
           "bvc1", "bvc2", "gnw", "gnb", "gnms", "lnw", "lnb"}



def _chunk_widths(S_t):
    """Split S_t (multiple of 128) into ceil(S_t/512) balanced widths,
    each a multiple of 128 and <= 512."""
    nch = math.ceil(S_t / 512)
    nsub = S_t // 128
    base = nsub // nch
    rem = nsub - base * nch
    return [(base + (1 if i < rem else 0)) * 128 for i in range(nch)]


def _build_nc(T, S_list, N_pad, G):
    assert G % 2 == 0, "graphs per core must be even (f32r even-N rule)"
    """Emit the per-core Bass program. T node tiles of 128, S_list[t] edge
    slots per node tile (mult of 128), N_pad = T*128 nodes, G graphs/core."""
    nc = bass.Bass()
    dp = nc.declare_dram_parameter
    S_list = list(S_list)
    S_off = [0]
    for s_ in S_list:
        S_off.append(S_off[-1] + s_)
    Stot = S_off[-1]
    S_max = max(S_list)
    KSUB = S_max // 128
    NCH = math.ceil(N_pad / 512)

    sss_e = dp("sss", [DS, 3 * Stot], FR, isOutput=False)  # si|sj|se per chunk
    vvv_e = dp("vvv", [80, Stot], FR, isOutput=False)      # vi@0, vj@32, ve@64
    oh_e = dp("oh", [Stot, 128], FR, isOutput=False)     # [edge, node] one-hot
    ohT_e = dp("ohT", [T * 128, S_max], FR, isOutput=False)  # [node, edge] one-hot

    s_nd = dp("s_n", [DS, N_pad], FR, isOutput=False)
    v_nd = dp("v_n", [DV, N_pad], FR, isOutput=False)
    sclsb_nd = dp("sclsb_n", [DS, N_pad], FR, isOutput=False)
    vclsb_nd = dp("vclsb_n", [DV, N_pad], FR, isOutput=False)
    gh_oh = dp("gh_oh", [N_pad, 128], FR, isOutput=False)
    gh_ohT = dp("gh_ohT", [128, N_pad], FR, isOutput=False)
    invc_d = dp("invc", [128, 1], F32, isOutput=False)
    scls_d = dp("scls_g", [DS, G], FR, isOutput=False)
    vcls_d = dp("vcls_g", [DV, G], FR, isOutput=False)
    wd = {name: dp(name, shp, FR if name.startswith("w") else F32, isOutput=False)
          for name, shp in _WSHAPES}

    s_out_e = dp("s_out", [DS, N_pad], FR, isOutput=True)
    v_out_e = dp("v_out", [DV, N_pad], FR, isOutput=True)
    scls_out_e = dp("scls_out", [DS, G], F32, isOutput=True)
    vcls_out_e = dp("vcls_out", [DV, G], F32, isOutput=True)

    with nc.allow_low_precision(reason="float32r compute"), \
         tile.TileContext(nc) as tc, contextlib.ExitStack() as ctx:
        ep = ctx.enter_context  # shorthand
        cst = ep(tc.tile_pool(name="cst", bufs=1))
        per = ep(tc.tile_pool(name="per", bufs=1))
        ectx = contextlib.ExitStack()   # edge-stage pools, freed before node stage
        eep = ectx.enter_context
        epin = eep(tc.tile_pool(name="epin", bufs=3))
        epu = eep(tc.tile_pool(name="epu", bufs=5))
        eph = eep(tc.tile_pool(name="eph", bufs=3))
        uhp = eep(tc.tile_pool(name="uhp", bufs=9))
        epm = eep(tc.tile_pool(name="epm", bufs=3))
        epm1 = eep(tc.tile_pool(name="epm1", bufs=1))
        bufp = eep(tc.tile_pool(name="bufp", bufs=KSUB + 6))
        ohp = eep(tc.tile_pool(name="ohp", bufs=1))
        wmp = eep(tc.tile_pool(name="wmp", bufs=3))
        # PSUM: 8 banks total -> pb(3) + ptr(2) + pz(1) + pacc(2)
        pb = ep(tc.tile_pool(name="pb", bufs=4, space="PSUM"))
        ptr = ep(tc.tile_pool(name="ptr", bufs=2, space="PSUM"))
        pzp = ep(tc.tile_pool(name="pz", bufs=1, space="PSUM"))
        pacc = ep(tc.tile_pool(name="pacc", bufs=1, space="PSUM"))

        def PB(p_, f_):
            return pb.tile([p_, f_], F32, tag="pb", name="pbt")

        def PTR(p_, f_, dt_=F32):
            return ptr.tile([p_, f_], dt_, tag="ptr", name="ptrt")

        # ---- constants ----
        ident_f = cst.tile([128, 128], F32)
        make_identity(nc, ident_f[:])
        ident = cst.tile([128, 128], FR)
        nc.vector.tensor_copy(ident[:], ident_f[:])
        cscr = cst.tile([128, 1], F32)
        nc.vector.memset(cscr[:], 1.0)
        ones15 = cst.tile([15, 1], FR)
        nc.vector.tensor_copy(ones15[:], cscr[0:15, :])
        cscr1x = cst.tile([1, 128], F32)
        nc.vector.memset(cscr1x[:], 1.0)
        ones1x = cst.tile([1, 128], FR)
        nc.vector.tensor_copy(ones1x[:], cscr1x[:])
        cscr2 = cst.tile([128, 1], F32)
        nc.vector.memset(cscr2[:], 1.0 / DS)
        oo128 = cst.tile([128, 1], FR)
        nc.vector.tensor_copy(oo128[:], cscr2[:])
        eps5 = cst.tile([128, 1], F32)
        nc.vector.memset(eps5[:], 1e-5)

        W = {}
        for name in _WSMALL:
            W[name] = cst.tile(list(wd[name].shape), FR if name.startswith("w") else F32,
                               tag=f"w_{name}", name=f"w_{name}")
            nc.sync.dma_start(out=W[name][:], in_=wd[name][:])

        def ksplit(name, splits):
            """Load dram weight rows into separate tiles per K-chunk."""
            tiles, r0 = [], 0
            for kk in splits:
                t_ = cst.tile([kk, wd[name].shape[1]], FR, tag=f"{name}_{r0}", name=f"{name}_{r0}")
                nc.sync.dma_start(out=t_[:], in_=wd[name][r0:r0 + kk, :])
                tiles.append(t_)
                r0 += kk
            return tiles

        wn1_k = ksplit("wn1", [128])
        we1_k = ksplit("we1", [128])
        wa1v = cst.tile([80, MA], FR)
        nc.sync.dma_start(out=wa1v[:], in_=wd["wa1v"][:])
        wnev = cst.tile([80, 2 * DU], FR)
        nc.sync.dma_start(out=wnev[:], in_=wd["wnev"][:])
        wf_k = ksplit("wf", [128] * 6)
        wm2_k = ksplit("wm2", [128, 128])
        wa1_k = ksplit("wa1", [128, 128, 128])  # si,sj,se rows
        wa2_k = ksplit("wa2", [128, 128, 128, 128, 60])
        wsd1_k = ksplit("wsd1", [128, 128, 128])
        wsc1_k = ksplit("wsc1", [128, 128, 128])
        wvd1_k = ksplit("wvd1", [15, 15, 15])
        wvc1_k = ksplit("wvc1", [15, 15, 15])
        wg1_k = ksplit("wg1", [1, 1, 1, 128])

        invc = cst.tile([128, 1], F32)
        nc.sync.dma_start(out=invc[:], in_=invc_d[:])
        scls_s = cst.tile([DS, G], FR)
        nc.sync.dma_start(out=scls_s[:], in_=scls_d[:])
        vcls_s = cst.tile([DV, G], FR)
        nc.sync.dma_start(out=vcls_s[:], in_=vcls_d[:])

        # ---- persistent buffers ----
        agg_sb = [per.tile([128, 144], FR, tag=f"agg{t}", name=f"agg{t}") for t in range(T)]
        sagg_fn = per.tile([DS, N_pad], FR)
        vagg_fn = per.tile([DV, N_pad], FR)
        x_fn = per.tile([DS, N_pad], FR)       # x, later overwritten by xc
        sout_fn = per.tile([DS, N_pad], FR)
        vout_fn = per.tile([16, N_pad], FR)
        mean_gf = per.tile([128, 128], FR)
        nm_g = per.tile([128, 144], FR)
        em_g = per.tile([128, 144], FR)

        # ================= EDGE STAGE =================
        for t in range(T):
            S_t = S_list[t]
            KSUB_t = S_t // 128
            oh_t = ohp.tile([128, KSUB, 128], FR, tag="oh", name="oh_t")
            nc.sync.dma_start(
                out=oh_t[:, 0:KSUB_t, :],
                in_=oh_e[S_off[t]:S_off[t + 1], :].rearrange(
                    "(k p) n -> p k n", p=128))
            ohT_t = ohp.tile([128, S_max], FR, tag="ohT", name="ohT_t")
            nc.sync.dma_start(out=ohT_t[:, 0:S_t],
                              in_=ohT_e[t * 128:(t + 1) * 128, 0:S_t])

            psz = pzp.tile([128, 4], F32, tag="pz")
            bufs_t = []
            widths = _chunk_widths(S_t)
            w_off = [0]
            for w_ in widths:
                w_off.append(w_off[-1] + w_)
            for ci, cw in enumerate(widths):
                e0 = S_off[t] + w_off[ci]
                sss_c = epin.tile([DS, 3 * cw], FR, tag="sss")
                nc.sync.dma_start(out=sss_c[:], in_=sss_e[:, 3 * e0:3 * e0 + 3 * cw])
                vvv_c = epin.tile([80, cw], FR, tag="vvv")
                nc.sync.dma_start(out=vvv_c[:], in_=vvv_e[:, e0:e0 + cw])
                si_c, sj_c, se_c = (sss_c[:, 0:cw], sss_c[:, cw:2 * cw],
                                    sss_c[:, 2 * cw:3 * cw])
                vi_c, vj_c, ve_c = (vvv_c[0:15, :], vvv_c[32:47, :],
                                    vvv_c[64:79, :])

                # interleaved emission: attn m-chunks woven between u-MLP
                # stages so PE always has independent matmuls to run while
                # ACT processes silus.
                def u_hid(s_in, v_in, v_w1, w1k, b1, m):
                    ph = PB(128, cw)
                    nc.tensor.matmul(ph[:], _r(w1k[0][:, 128 * m:128 * (m + 1)]),
                                     _r(s_in), start=True, stop=False)
                    nc.tensor.matmul(ph[:], _r(v_w1[:, 128 * m:128 * (m + 1)]),
                                     _r(v_in), start=False, stop=True)
                    h = uhp.tile([128, cw], FR, tag="uh", name="h")
                    nc.scalar.activation(h[:], ph[:], AF.Silu, bias=b1[:, m:m + 1])
                    return h

                praw = PB(4, cw)

                def attn_m(m):
                    mw = min(128, MA - 128 * m)
                    ph = PB(128, cw)
                    ins = [(wa1_k[0][:, 128 * m:128 * m + mw], si_c),
                           (wa1_k[1][:, 128 * m:128 * m + mw], sj_c),
                           (wa1_k[2][:, 128 * m:128 * m + mw], se_c),
                           (wa1v[:, 128 * m:128 * m + mw], vvv_c[:])]
                    for ki, (wt, xin) in enumerate(ins):
                        nc.tensor.matmul(ph[:mw, :], _r(wt), _r(xin),
                                         start=(ki == 0), stop=(ki == 3))
                    h = eph.tile([128, cw], FR, tag="ah", name="ah")
                    nc.scalar.activation(h[:mw, :], ph[:mw, :], AF.Silu,
                                         bias=W["ba1"][:mw, m:m + 1])
                    nc.tensor.matmul(praw[:], _r(wa2_k[m][:]), _r(h[:mw, :]),
                                     start=(m == 0), stop=(m == 4))

                attn_m(0)
                ui0 = u_hid(si_c, vi_c, wnev[0:15, :], wn1_k, W["bn1"], 0)
                ui1 = u_hid(si_c, vi_c, wnev[0:15, :], wn1_k, W["bn1"], 1)
                attn_m(1)
                uj0 = u_hid(sj_c, vj_c, wnev[32:47, :], wn1_k, W["bn1"], 0)
                uj1 = u_hid(sj_c, vj_c, wnev[32:47, :], wn1_k, W["bn1"], 1)
                attn_m(2)
                ue0 = u_hid(se_c, ve_c, wnev[64:79, :], we1_k, W["be1"], 0)
                ue1 = u_hid(se_c, ve_c, wnev[64:79, :], we1_k, W["be1"], 1)
                attn_m(3)
                uhids = [ui0, ui1, uj0, uj1, ue0, ue1]
                attn_m(4)
                er = epm1.tile([4, cw], FR, tag="er")
                nc.scalar.activation(er[:], praw[:], AF.Exp, bias=W["ba2"][:, 0:1])

                # msg L1 folded over u-hiddens: Wf = W2_{node,edge} @ W1_msg chunks
                mh = []
                for m in range(2):
                    ph = PB(128, cw)
                    for ki, hh in enumerate(uhids):
                        nc.tensor.matmul(ph[:], _r(wf_k[ki][:, 128 * m:128 * (m + 1)]),
                                         _r(hh[:]), start=(ki == 0), stop=(ki == 5))
                    h = uhp.tile([128, cw], FR, tag="uh", name="mhh")
                    nc.scalar.activation(h[:], ph[:], AF.Silu, bias=W["bm1"][:, m:m + 1])
                    mh.append(h)
                pm = PB(128, cw)
                nc.tensor.matmul(pm[:], _r(wm2_k[0][:]), _r(mh[0][:]),
                                 start=True, stop=False)
                nc.tensor.matmul(pm[:], _r(wm2_k[1][:]), _r(mh[1][:]),
                                 start=False, stop=True)
                msg = epm.tile([128, cw], FR, tag="msg")
                nc.vector.tensor_scalar_add(msg[:], pm[:], W["bm2"][:, 0:1])

                # coeff = mlp2(msg) 128->64->1
                pch = PB(64, cw)
                nc.tensor.matmul(pch[:], _r(W["ws1"][:]), _r(msg[:]),
                                 start=True, stop=True)
                chh = eph.tile([64, cw], FR, tag="ch")
                nc.scalar.activation(chh[:], pch[:], AF.Silu, bias=W["bs1"][:, 0:1])
                pco = PB(1, cw)
                nc.tensor.matmul(pco[:], _r(W["ws2"][:]), _r(chh[:]),
                                 start=True, stop=True)
                coeff = epm1.tile([1, cw], FR, tag="cf")
                nc.vector.tensor_scalar_add(coeff[:], pco[:], W["bs2"][:, 0:1])

                # v_msg = v_edge * coeff (coeff broadcast to 15 partitions)
                pcb = PB(15, cw)
                nc.tensor.matmul(pcb[:], _r(ones1x[0:1, 0:15]), _r(coeff[:]),
                                 start=True, stop=True)
                vmsg_c = epm1.tile([16, cw], FR, tag="vmsgc")
                nc.vector.tensor_mul(vmsg_c[0:15, :], ve_c, pcb[:])

                # transpose msg/vmsg/er to [edge_p, feat]; z aggregation
                for kk in range(cw // 128):
                    kg = w_off[ci] // 128 + kk
                    sl = slice(128 * kk, 128 * (kk + 1))
                    buf = bufp.tile([128, 147], FR, tag="buf")
                    bufs_t.append(buf)
                    ptm = PTR(128, 128, FR)
                    nc.tensor.transpose(ptm[:], msg[:, sl], ident[:])
                    nc.vector.tensor_copy(buf[:, 0:128], ptm[:])
                    ptv = PTR(128, 16, FR)
                    nc.tensor.transpose(ptv[:], vmsg_c[:, sl], ident[0:16, 0:16])
                    nc.vector.tensor_copy(buf[:, 128:143], ptv[:, 0:15])
                    pte = PTR(128, 4, FR)
                    nc.tensor.transpose(pte[:], er[:, sl], ident[0:4, 0:4])
                    nc.vector.tensor_copy(buf[:, 143:147], pte[:])
                    nc.tensor.matmul(psz[:], _r(oh_t[:, kg, :]), _r(buf[:, 143:147]),
                                     start=(kg == 0), stop=(kg == KSUB_t - 1))

            # z -> 1/max(z, eps)
            rz = epm.tile([128, 4], FR, tag="rz")
            nc.vector.tensor_scalar(out=rz[:], in0=psz[:], scalar1=1e-30,
                                    scalar2=4.0, op0=OP.max, op1=OP.mult)
            nc.vector.reciprocal(rz[:], rz[:])

            # pass B: broadcast 1/z to edges, weight msgs, aggregate
            pag = pacc.tile([128, 144], F32, tag="pacc")
            for kg in range(KSUB_t):
                buf = bufs_t[kg]
                pzb = PTR(128, 4)
                nc.tensor.matmul(pzb[:], _r(ohT_t[:, 128 * kg:128 * (kg + 1)]),
                                 _r(rz[:]), start=True, stop=True)
                scr = wmp.tile([128, 4], F32, tag="scr")
                wT = wmp.tile([128, 1], F32, tag="wT")
                nc.vector.tensor_mul(scr[:], buf[:, 143:147], pzb[:])
                nc.vector.reduce_sum(wT[:], scr[:], axis=mybir.AxisListType.X)
                wmsg = wmp.tile([128, 144], FR, tag="wmsg")
                nc.vector.tensor_scalar_mul(wmsg[:], buf[:, 0:144], wT[:, 0:1])
                nc.tensor.matmul(pag[:], _r(oh_t[:, kg, :]), _r(wmsg[:]),
                                 start=(kg == 0), stop=(kg == KSUB_t - 1))
            nc.vector.tensor_copy(agg_sb[t][:], pag[:])
            pta = PTR(128, 128, FR)
            nc.tensor.transpose(pta[:], agg_sb[t][:, 0:128], ident[:])
            nc.vector.tensor_copy(sagg_fn[:, 128 * t:128 * (t + 1)], pta[:])
            ptb = PTR(16, 128, FR)
            nc.tensor.transpose(ptb[:], agg_sb[t][:, 128:144], ident[:])
            nc.vector.tensor_copy(vagg_fn[:, 128 * t:128 * (t + 1)], ptb[0:15, :])

        # ================= NODE STAGE =================
        ectx.close()
        nsp = ep(tc.tile_pool(name="nsp", bufs=2))
        nsp1 = ep(tc.tile_pool(name="nsp1", bufs=1))
        gp = ep(tc.tile_pool(name="gp", bufs=1))
        ghoh = gp.tile([128, T * 128], FR)
        for t in range(T):
            nc.sync.dma_start(out=ghoh[:, 128 * t:128 * (t + 1)],
                              in_=gh_oh[128 * t:128 * (t + 1), :])
        ghohT = gp.tile([128, N_pad], FR)
        nc.sync.dma_start(out=ghohT[:], in_=gh_ohT[:])
        for ci in range(NCH):
            n0 = ci * 512
            cw = min(512, N_pad - n0)
            sl = slice(n0, n0 + cw)
            s_c = nsp.tile([DS, cw], FR, tag="s_c")
            nc.sync.dma_start(out=s_c[:], in_=s_nd[:, sl])
            scb_c = nsp.tile([DS, cw], FR, tag="scb")
            nc.sync.dma_start(out=scb_c[:], in_=sclsb_nd[:, sl])
            v_c = nsp1.tile([DV, cw], FR, tag="v_c")
            nc.sync.dma_start(out=v_c[:], in_=v_nd[:, sl])
            vcb_c = nsp1.tile([DV, cw], FR, tag="vcb")
            nc.sync.dma_start(out=vcb_c[:], in_=vclsb_nd[:, sl])

            # s_delta MLP 384->128->128 ; x = s + s_delta
            ph = PB(128, cw)
            for ki, xin in enumerate([s_c[:], sagg_fn[:, sl], scb_c[:]]):
                nc.tensor.matmul(ph[:], _r(wsd1_k[ki][:]), _r(xin),
                                 start=(ki == 0), stop=(ki == 2))
            h = nsp.tile([128, cw], FR, tag="nh")
            nc.scalar.activation(h[:], ph[:], AF.Silu, bias=W["bsd1"][:, 0:1])
            po = PB(128, cw)
            nc.tensor.matmul(po[:], _r(W["wsd2"][:]), _r(h[:]), start=True, stop=True)
            nc.vector.scalar_tensor_tensor(
                out=x_fn[:, sl], in0=po[:], scalar=W["bsd2"][:, 0:1],
                in1=s_c[:], op0=OP.add, op1=OP.add)

            # v_delta MLP 45->15->15
            pvh = PB(15, cw)
            for ki, xin in enumerate([v_c[:], vagg_fn[:, sl], vcb_c[:]]):
                nc.tensor.matmul(pvh[:], _r(wvd1_k[ki][:]), _r(xin),
                                 start=(ki == 0), stop=(ki == 2))
            vh = nsp1.tile([15, cw], FR, tag="vh")
            nc.scalar.activation(vh[:], pvh[:], AF.Silu, bias=W["bvd1"][:, 0:1])
            pvo = PB(15, cw)
            nc.tensor.matmul(pvo[:], _r(W["wvd2"][:]), _r(vh[:]), start=True, stop=True)
            vd_c = nsp1.tile([DV, cw], FR, tag="vd_c")
            nc.vector.tensor_scalar_add(vd_c[:], pvo[:], W["bvd2"][:, 0:1])

            # gate: concat(|v|, |vd|, cos, s) -> 128 -> 1 -> sigmoid
            t2 = nsp1.tile([15, cw], FR, tag="t2")
            nc.vector.tensor_mul(t2[:], v_c[:], v_c[:])
            pvn = PB(1, cw)
            nc.tensor.matmul(pvn[:], _r(ones15[:]), _r(t2[:]), start=True, stop=True)
            vn = nsp1.tile([1, cw], FR, tag="vn")
            nc.scalar.activation(vn[:], pvn[:], AF.Sqrt)
            t3 = nsp1.tile([15, cw], FR, tag="t3")
            nc.vector.tensor_mul(t3[:], vd_c[:], vd_c[:])
            pdn = PB(1, cw)
            nc.tensor.matmul(pdn[:], _r(ones15[:]), _r(t3[:]), start=True, stop=True)
            dn = nsp1.tile([1, cw], FR, tag="dn")
            nc.scalar.activation(dn[:], pdn[:], AF.Sqrt)
            t4 = nsp1.tile([15, cw], FR, tag="t4")
            nc.vector.tensor_mul(t4[:], v_c[:], vd_c[:])
            pdo = PB(1, cw)
            nc.tensor.matmul(pdo[:], _r(ones15[:]), _r(t4[:]), start=True, stop=True)
            den = nsp1.tile([1, cw], F32, tag="den")
            nc.vector.tensor_mul(den[:], vn[:], dn[:])
            nc.vector.tensor_scalar_add(den[:], den[:], 1e-6)
            nc.vector.reciprocal(den[:], den[:])
            cosn = nsp1.tile([1, cw], FR, tag="cosn")
            nc.vector.tensor_mul(cosn[:], pdo[:], den[:])
            pgh = PB(128, cw)
            nc.tensor.matmul(pgh[:], _r(wg1_k[0][:]), _r(vn[:]), start=True, stop=False)
            nc.tensor.matmul(pgh[:], _r(wg1_k[1][:]), _r(dn[:]), start=False, stop=False)
            nc.tensor.matmul(pgh[:], _r(wg1_k[2][:]), _r(cosn[:]), start=False, stop=False)
            nc.tensor.matmul(pgh[:], _r(wg1_k[3][:]), _r(s_c[:]),
                             start=False, stop=True)
            gh = nsp.tile([128, cw], FR, tag="nh")
            nc.scalar.activation(gh[:], pgh[:], AF.Silu, bias=W["bg1"][:, 0:1])
            pgo = PB(1, cw)
            nc.tensor.matmul(pgo[:], _r(W["wg2"][:]), _r(gh[:]), start=True, stop=True)
            gate = nsp1.tile([1, cw], FR, tag="gate")
            nc.scalar.activation(gate[:], pgo[:], AF.Sigmoid, bias=W["bg2"][:, 0:1])
            pgb = PB(15, cw)
            nc.tensor.matmul(pgb[:], _r(ones1x[0:1, 0:15]), _r(gate[:]),
                             start=True, stop=True)
            nc.vector.tensor_mul(vout_fn[0:15, sl], vd_c[:], pgb[:])
            nc.vector.tensor_add(vout_fn[0:15, sl], vout_fn[0:15, sl], v_c[:])

        # ---- GraphNorm (one-pass stats: aggregate [x | x^2] together) ----
        pgs = pacc.tile([128, 256], F32, tag="pacc", name="pgs")
        for t in range(T):
            sq = nsp.tile([128, 128], FR, tag="sq")
            nc.vector.tensor_mul(sq[:], x_fn[:, 128 * t:128 * (t + 1)],
                                 x_fn[:, 128 * t:128 * (t + 1)])
            ptx = PTR(128, 128, FR)
            nc.tensor.transpose(ptx[:], x_fn[:, 128 * t:128 * (t + 1)], ident[:])
            ptq = PTR(128, 128, FR)
            nc.tensor.transpose(ptq[:], sq[:], ident[:])
            xts = nsp.tile([128, 256], FR, tag="xts")
            nc.vector.tensor_copy(xts[:, 0:128], ptx[:])
            nc.vector.tensor_copy(xts[:, 128:256], ptq[:])
            nc.tensor.matmul(pgs[:], _r(ghoh[:, 128 * t:128 * (t + 1)]), _r(xts[:]),
                             start=(t == 0), stop=(t == T - 1))
        nc.vector.tensor_scalar_mul(mean_gf[:], pgs[:, 0:128], invc[:, 0:1])
        ex2_gf = per.tile([128, 128], FR)
        nc.vector.tensor_scalar_mul(ex2_gf[:], pgs[:, 128:256], invc[:, 0:1])
        for ci in range(NCH):
            n0 = ci * 512
            cw = min(512, N_pad - n0)
            sl = slice(n0, n0 + cw)
            pmb = PB(128, cw)
            nc.tensor.matmul(pmb[:], _r(mean_gf[:]), _r(ghohT[:, sl]),
                             start=True, stop=True)
            pxb = PB(128, cw)
            nc.tensor.matmul(pxb[:], _r(ex2_gf[:]), _r(ghohT[:, sl]),
                             start=True, stop=True)
            tmb = nsp.tile([128, cw], F32, tag="nh")      # gms * mean_b
            nc.vector.tensor_scalar_mul(tmb[:], pmb[:], W["gnms"][:, 0:1])
            qb = nsp.tile([128, cw], F32, tag="qb")       # 2*mean_b - tmb
            nc.vector.scalar_tensor_tensor(out=qb[:], in0=pmb[:], scalar=2.0,
                                           in1=tmb[:], op0=OP.mult, op1=OP.subtract)
            vb = nsp.tile([128, cw], F32, tag="vb")       # (2g-g^2) mean^2
            nc.vector.tensor_mul(vb[:], tmb[:], qb[:])
            nc.vector.tensor_sub(x_fn[:, sl], x_fn[:, sl], tmb[:])   # xc in place
            varb = nsp.tile([128, cw], F32, tag="vrb")
            nc.vector.tensor_tensor(out=varb[:], in0=pxb[:], in1=vb[:],
                                    op=OP.subtract)
            nc.scalar.activation(varb[:], varb[:], AF.Sqrt, bias=eps5[:, 0:1])
            nc.vector.reciprocal(varb[:], varb[:])
            nc.vector.tensor_mul(sout_fn[:, sl], x_fn[:, sl], varb[:])
            nc.vector.tensor_scalar(out=sout_fn[:, sl], in0=sout_fn[:, sl],
                                    scalar1=W["gnw"][:, 0:1], scalar2=W["gnb"][:, 0:1],
                                    op0=OP.mult, op1=OP.add)
            nc.sync.dma_start(out=s_out_e[:, sl], in_=sout_fn[:, sl])
            nc.sync.dma_start(out=v_out_e[:, sl], in_=vout_fn[0:15, sl])

        # ---- CLS stage ----
        pnm = pacc.tile([128, 144], F32, tag="pacc")
        pem = pacc.tile([128, 144], F32, tag="pacc")
        for t in range(T):
            so_t = nsp.tile([128, 144], FR, tag="sot")
            ptx = PTR(128, 128, FR)
            nc.tensor.transpose(ptx[:], sout_fn[:, 128 * t:128 * (t + 1)], ident[:])
            nc.vector.tensor_copy(so_t[:, 0:128], ptx[:])
            ptv = PTR(128, 16, FR)
            nc.tensor.transpose(ptv[:], vout_fn[:, 128 * t:128 * (t + 1)],
                                ident[0:16, 0:16])
            nc.vector.tensor_copy(so_t[:, 128:144], ptv[:])
            nc.tensor.matmul(pnm[:], _r(ghoh[:, 128 * t:128 * (t + 1)]), _r(so_t[:]),
                             start=(t == 0), stop=(t == T - 1))
            nc.tensor.matmul(pem[:], _r(ghoh[:, 128 * t:128 * (t + 1)]),
                             _r(agg_sb[t][:]), start=(t == 0), stop=(t == T - 1))
        nc.vector.tensor_scalar_mul(nm_g[:], pnm[:], invc[:, 0:1])
        nc.vector.tensor_scalar_mul(em_g[:], pem[:], invc[:, 0:1])

        def tr_fg(src_ap, P, Fw, tag):
            pt = PTR(Fw, P, FR)
            nc.tensor.transpose(pt[:], src_ap, ident[0:P, 0:P])
            dst = nsp.tile([Fw, P], FR, tag=tag)
            nc.vector.tensor_copy(dst[:], pt[:])
            return dst

        snm_fg = tr_fg(nm_g[0:G, 0:128], G, 128, "snm")
        sem_fg = tr_fg(em_g[0:G, 0:128], G, 128, "sem")
        vnm_fg = tr_fg(nm_g[0:G, 128:144], G, 16, "vnm")
        vem_fg = tr_fg(em_g[0:G, 128:144], G, 16, "vem")

        # s_cls update + layernorm over features
        ph = PB(128, G)
        for ki, xin in enumerate([scls_s[:], snm_fg[:], sem_fg[:]]):
            nc.tensor.matmul(ph[:], _r(wsc1_k[ki][:]), _r(xin),
                             start=(ki == 0), stop=(ki == 2))
        h = nsp.tile([128, G], FR, tag="nh")
        nc.scalar.activation(h[:], ph[:], AF.Silu, bias=W["bsc1"][:, 0:1])
        po = PB(128, G)
        nc.tensor.matmul(po[:], _r(W["wsc2"][:]), _r(h[:]), start=True, stop=True)
        yc = nsp.tile([128, G], FR, tag="yc")
        nc.vector.scalar_tensor_tensor(out=yc[:], in0=po[:], scalar=W["bsc2"][:, 0:1],
                                       in1=scls_s[:], op0=OP.add, op1=OP.add)
        pmu = PB(1, G)
        nc.tensor.matmul(pmu[:], _r(oo128[:]), _r(yc[:]), start=True, stop=True)
        mu = nsp.tile([1, G], FR, tag="mu")
        nc.vector.tensor_copy(mu[:], pmu[:])
        pmb = PB(128, G)
        nc.tensor.matmul(pmb[:], _r(ones1x[0:1, :]), _r(mu[:]), start=True, stop=True)
        ycc = nsp.tile([128, G], FR, tag="ycc")
        nc.vector.tensor_sub(ycc[:], yc[:], pmb[:])
        sqg = nsp.tile([128, G], FR, tag="sqg")
        nc.vector.tensor_mul(sqg[:], ycc[:], ycc[:])
        pvv = PB(1, G)
        nc.tensor.matmul(pvv[:], _r(oo128[:]), _r(sqg[:]), start=True, stop=True)
        rs = nsp.tile([1, G], FR, tag="rs")
        nc.scalar.activation(rs[:], pvv[:], AF.Sqrt, bias=eps5[0:1, 0:1])
        nc.vector.reciprocal(rs[:], rs[:])
        prb = PB(128, G)
        nc.tensor.matmul(prb[:], _r(ones1x[0:1, :]), _r(rs[:]), start=True, stop=True)
        sco = nsp.tile([128, G], F32, tag="sco")
        nc.vector.tensor_mul(sco[:], ycc[:], prb[:])
        nc.vector.tensor_scalar(out=sco[:], in0=sco[:], scalar1=W["lnw"][:, 0:1],
                                scalar2=W["lnb"][:, 0:1], op0=OP.mult, op1=OP.add)
        nc.sync.dma_start(out=scls_out_e[:], in_=sco[:])

        # v_cls update
        pvh = PB(15, G)
        for ki, xin in enumerate([vcls_s[:], vnm_fg[0:15, :], vem_fg[0:15, :]]):
            nc.tensor.matmul(pvh[:], _r(wvc1_k[ki][:]), _r(xin),
                             start=(ki == 0), stop=(ki == 2))
        vh = nsp.tile([15, G], FR, tag="vh")
        nc.scalar.activation(vh[:], pvh[:], AF.Silu, bias=W["bvc1"][:, 0:1])
        pvo = PB(15, G)
        nc.tensor.matmul(pvo[:], _r(W["wvc2"][:]), _r(vh[:]), start=True, stop=True)
        vco = nsp.tile([15, G], F32, tag="vco")
        nc.vector.scalar_tensor_tensor(out=vco[:], in0=pvo[:], scalar=W["bvc2"][:, 0:1],
                                       in1=vcls_s[:], op0=OP.add, op1=OP.add)
        nc.sync.dma_start(out=vcls_out_e[:], in_=vco[:])

    _split_waits(nc)
    return nc


def _prep(s, v, s_edge, v_edge, edge_index, s_cls, v_cls, batch, params):
    """Host-side index prep + sharding. Returns (meta, in_maps)."""
    s = np.asarray(s, np.float32)
    v = np.asarray(v, np.float32)
    s_edge = np.asarray(s_edge, np.float32)
    v_edge = np.asarray(v_edge, np.float32)
    s_cls = np.asarray(s_cls, np.float32)
    v_cls = np.asarray(v_cls, np.float32)
    ei = np.asarray(edge_index).astype(np.int64)
    batch_np = np.asarray(batch).astype(np.int64)
    row, col = ei[0], ei[1]
    N, E, B = s.shape[0], row.shape[0], s_cls.shape[0]
    G = B // C

    g_lo = np.arange(C, dtype=np.int64) * G
    n_lo = np.searchsorted(batch_np, g_lo).astype(np.int64)
    n_hi = np.append(n_lo[1:], N).astype(np.int64)
    n_cnt = n_hi - n_lo
    N_pad = 128 * math.ceil(n_cnt.max() / 128)
    T = N_pad // 128

    edge_core = np.searchsorted(n_hi, row, side="right")
    local_row = row - n_lo[edge_core]
    tile_of_edge = local_row // 128
    key = edge_core * T + tile_of_edge
    counts = np.bincount(key, minlength=C * T).reshape(C, T)
    S_list = np.maximum(512, 128 * np.ceil(counts.max(0) / 128).astype(np.int64))
    S_off = np.zeros(T + 1, np.int64)
    S_off[1:] = np.cumsum(S_list)
    Stot1 = int(S_off[-1])
    S_max = int(S_list.max())
    order = np.argsort(key, kind="stable")
    key_o = key[order]
    starts = np.zeros(C * T + 1, np.int64)
    starts[1:] = np.cumsum(counts.reshape(-1))
    rank = np.arange(E, dtype=np.int64) - starts[key_o]
    core_o = key_o // T
    tile_o = key_o % T
    gslot = core_o * Stot1 + S_off[tile_o] + rank
    row_o, col_o = row[order], col[order]
    lr_o = local_row[order]

    CTS = C * Stot1
    si_all = np.zeros((CTS, DS), np.float32)
    sj_all = np.zeros((CTS, DS), np.float32)
    se_all = np.zeros((CTS, DS), np.float32)
    vi_all = np.zeros((CTS, DV), np.float32)
    vj_all = np.zeros((CTS, DV), np.float32)
    ve_all = np.zeros((CTS, DV), np.float32)
    chunk_bounds = []
    for t in range(T):
        St = int(S_list[t])
        off = int(S_off[t])
        for w_ in _chunk_widths(St):
            chunk_bounds.append((off, off + w_))
            off += w_
    si_all[gslot] = s[row_o]
    sj_all[gslot] = s[col_o]
    se_all[gslot] = s_edge[order]
    vi_all[gslot] = v[row_o]
    vj_all[gslot] = v[col_o]
    ve_all[gslot] = v_edge[order]
    oh_all = np.zeros((CTS, 128), np.float32)
    oh_all[gslot, lr_o % 128] = 1.0
    ohT_all = np.zeros((C * T * 128, S_max), np.float32)
    ohT_all[key_o * 128 + lr_o % 128, rank] = 1.0

    cnt = np.bincount(batch_np, minlength=B).astype(np.float32)
    inv_cnt = 1.0 / np.maximum(cnt, 1.0)

    p = params
    wn1, bn1, wn2, bn2 = [np.asarray(a, np.float32) for a in p["node"]]
    we1, be1, we2, be2 = [np.asarray(a, np.float32) for a in p["edge"]]
    wm1, bm1, wm2, bm2 = [np.asarray(a, np.float32) for a in p["msg"]]
    ws1, bs1, ws2, bs2 = [np.asarray(a, np.float32) for a in p["s2v"]]
    wsd1, bsd1, wsd2, bsd2 = [np.asarray(a, np.float32) for a in p["s_delta"]]
    wvd1, bvd1, wvd2, bvd2 = [np.asarray(a, np.float32) for a in p["v_delta"]]
    wg1, bg1, wg2, bg2 = [np.asarray(a, np.float32) for a in p["gate"]]
    wsc1, bsc1, wsc2, bsc2 = [np.asarray(a, np.float32) for a in p["s_cls"]]
    wvc1, bvc1, wvc2, bvc2 = [np.asarray(a, np.float32) for a in p["v_cls"]]
    aw1 = np.asarray(p["attn_W1"], np.float32)
    ab1 = np.asarray(p["attn_b1"], np.float32)
    aw2 = np.asarray(p["attn_W2"], np.float32)
    ab2 = np.asarray(p["attn_b2"], np.float32)
    Fdim = aw1.shape[1]
    wa1f = np.ascontiguousarray(np.transpose(aw1, (1, 0, 2)).reshape(Fdim, MA))
    wa1 = wa1f[0:384]
    wa1v = np.zeros((80, MA), np.float32)
    wa1v[0:15] = wa1f[384:399]
    wa1v[32:47] = wa1f[399:414]
    wa1v[64:79] = wa1f[414:429]
    wnev = np.zeros((80, 2 * DU), np.float32)
    wnev[0:15] = wn1[128:143]
    wnev[32:47] = wn1[128:143]
    wnev[64:79] = we1[128:143]
    wa2 = np.zeros((MA, HH), np.float32)
    for hh in range(HH):
        wa2[hh * HID_A:(hh + 1) * HID_A, hh] = aw2[hh]
    ba1 = ab1.reshape(MA)
    ba1_pad = np.zeros((128, 5), np.float32)
    for m in range(5):
        mw = min(128, MA - 128 * m)
        ba1_pad[:mw, m] = ba1[128 * m:128 * m + mw]
    bm1_eff = bm1 + bn2 @ wm1[0:DU] + bn2 @ wm1[DU:2 * DU] + be2 @ wm1[2 * DU:3 * DU]
    wf = np.concatenate([wn2 @ wm1[0:DU], wn2 @ wm1[DU:2 * DU],
                         we2 @ wm1[2 * DU:3 * DU]], axis=0)  # [768, 256]

    weights = {
        "wn1": wn1, "wn2": wn2, "we1": we1, "we2": we2,
        "wm1": wm1, "wf": wf, "wm2": wm2, "ws1": ws1, "ws2": ws2,
        "wa1": wa1, "wa1v": wa1v, "wnev": wnev, "wa2": wa2,
        "wsd1": wsd1, "wsd2": wsd2,
        "wvd1": wvd1, "wvd2": wvd2, "wg1": wg1, "wg2": wg2,
        "wsc1": wsc1, "wsc2": wsc2, "wvc1": wvc1, "wvc2": wvc2,
        "bn1": np.ascontiguousarray(np.stack([bn1[:128], bn1[128:]], 1)),
        "be1": np.ascontiguousarray(np.stack([be1[:128], be1[128:]], 1)),
        "bm1": np.ascontiguousarray(np.stack([bm1_eff[:128], bm1_eff[128:]], 1)),
        "bm2": bm2[:, None], "bs1": bs1[:, None], "bs2": bs2[:, None],
        "ba1": ba1_pad, "ba2": ab2[:, None],
        "bsd1": bsd1[:, None], "bsd2": bsd2[:, None],
        "bvd1": bvd1[:, None], "bvd2": bvd2[:, None],
        "bg1": bg1[:, None], "bg2": bg2[:, None],
        "bsc1": bsc1[:, None], "bsc2": bsc2[:, None],
        "bvc1": bvc1[:, None], "bvc2": bvc2[:, None],
        "gnw": np.asarray(p["gn_weight"], np.float32)[:, None],
        "gnb": np.asarray(p["gn_bias"], np.float32)[:, None],
        "gnms": np.asarray(p["gn_mean_scale"], np.float32)[:, None],
        "lnw": np.asarray(p["ln_w"], np.float32)[:, None],
        "lnb": np.asarray(p["ln_b"], np.float32)[:, None],
    }
    weights = {k: np.ascontiguousarray(a) for k, a in weights.items()}

    in_maps = []
    for c in range(C):
        lo, hi, ncn = int(n_lo[c]), int(n_hi[c]), int(n_cnt[c])
        esl = slice(c * Stot1, (c + 1) * Stot1)
        bloc = batch_np[lo:hi] - c * G
        siT = si_all[esl].T
        sjT = sj_all[esl].T
        seT = se_all[esl].T
        cols = []
        for (a, b) in chunk_bounds:
            cols += [siT[:, a:b], sjT[:, a:b], seT[:, a:b]]
        sss = np.ascontiguousarray(np.concatenate(cols, axis=1))
        vvv = np.zeros((80, Stot1), np.float32)
        vvv[0:15] = vi_all[esl].T
        vvv[32:47] = vj_all[esl].T
        vvv[64:79] = ve_all[esl].T
        m = {
            "sss": sss,
            "vvv": vvv,
            "oh": np.ascontiguousarray(oh_all[esl]),
            "ohT": np.ascontiguousarray(ohT_all[c * T * 128:(c + 1) * T * 128]),
        }
        for name, dat, Pdim in [
            ("s_n", s[lo:hi], DS), ("v_n", v[lo:hi], DV),
            ("sclsb_n", s_cls[batch_np[lo:hi]], DS),
            ("vclsb_n", v_cls[batch_np[lo:hi]], DV),
        ]:
            a = np.zeros((Pdim, N_pad), np.float32)
            a[:, :ncn] = dat.T
            m[name] = a
        goh = np.zeros((N_pad, 128), np.float32)
        goh[np.arange(ncn), bloc] = 1.0
        m["gh_oh"] = goh
        gohT = np.zeros((128, N_pad), np.float32)
        gohT[bloc, np.arange(ncn)] = 1.0
        m["gh_ohT"] = gohT
        ic = np.ones((128, 1), np.float32)
        ic[:G, 0] = inv_cnt[c * G:(c + 1) * G]
        m["invc"] = ic
        m["scls_g"] = np.ascontiguousarray(s_cls[c * G:(c + 1) * G].T)
        m["vcls_g"] = np.ascontiguousarray(v_cls[c * G:(c + 1) * G].T)
        m.update(weights)
        in_maps.append(m)

    meta = dict(T=T, S_list=tuple(int(s) for s in S_list), N_pad=N_pad, G=G,
                n_lo=n_lo, n_hi=n_hi, N=N, B=B)
    return meta, in_maps


def kernel(s, v, s_edge, v_edge, edge_index, s_cls, v_cls, batch, params):
    meta, in_maps = _prep(s, v, s_edge, v_edge, edge_index, s_cls, v_cls,
                          batch, params)
    key = (meta["T"], meta["S_list"], meta["N_pad"], meta["G"], USE_F32R)
    if key not in _BUILD_CACHE:
        _BUILD_CACHE[key] = _build_nc(meta["T"], meta["S_list"], meta["N_pad"],
                                      meta["G"])
    nc = _BUILD_CACHE[key]
    res = run_bass_kernel_spmd(nc, in_maps, list(range(C)))

    N, B, G = meta["N"], meta["B"], meta["G"]
    s_out = np.zeros((N, DS), np.float32)
    v_out = np.zeros((N, DV), np.float32)
    scls_out = np.zeros((B, DS), np.float32)
    vcls_out = np.zeros((B, DV), np.float32)
    for c in range(C):
        lo, hi = int(meta["n_lo"][c]), int(meta["n_hi"][c])
        ncn = hi - lo
        r = res.results[c]
        s_out[lo:hi] = r["s_out"][:, :ncn].T
        v_out[lo:hi] = r["v_out"][:, :ncn].T
        scls_out[c * G:(c + 1) * G] = r["scls_out"].T
        vcls_out[c * G:(c + 1) * G] = r["vcls_out"].T
    return s_out, v_out, scls_out, vcls_out


# revision 28
# speedup vs baseline: 1.0371x; 1.0371x over previous
"""Trainium2 Bass kernel for nn_AllInOneLayer (GNN message-passing layer).

Sharding: data-parallel over graphs. 8 cores, B/8 graphs each; nodes/edges
of a core's graphs are contiguous (batch is sorted). Edges are assigned to
the core that owns their DESTINATION node (row = edge_index[0]); within a
core they are grouped per 128-node tile with a uniform padded slot count S,
so the segment softmax and scatter-adds become core-local PE matmuls with
host-built one-hot matrices. All float math runs on the device; the host
only does index manipulation / gathers / layout packing.
"""

import contextlib
import math

import numpy as np
import ml_dtypes
_bf16 = ml_dtypes.bfloat16

import concourse.bass as bass
import concourse.tile as tile
from concourse import mybir
from concourse.bass_utils import run_bass_kernel_spmd
from concourse.masks import make_identity

AF = mybir.ActivationFunctionType
OP = mybir.AluOpType
F32 = mybir.dt.float32
F32R = mybir.dt.float32r
BF16 = mybir.dt.bfloat16

C = 8           # cores
DS, DV, DU, HH = 128, 15, 128, 4
HID_A = DS + DV           # 143, attn hidden per head
MA = HH * HID_A           # 572, stacked attn hidden
USE_F32R = True           # relaxed-precision matmuls (4x faster, ~1e-4 rel err)
FR = F32R if USE_F32R else F32   # dtype for every matmul-feeding tensor

_BUILD_CACHE = {}


def _r(ap):
    """Matmul operands already carry FR dtype; kept for call-site clarity."""
    return ap


def _split_waits(nc, limit=1):
    """This toolchain's walrus accepts at most one sync-wait per instruction;
    move excess waits onto preceding same-engine NoOps."""
    for f in nc.m.functions:
        for bb in f.blocks:
            new = []
            for inst in bb.instructions:
                si = inst.sync_info
                waits = list(si.on_wait) if si else []
                if len(waits) > limit:
                    chunks = [waits[i:i + limit] for i in range(0, len(waits), limit)]
                    for ch in chunks[:-1]:
                        nop = mybir.InstNoOp(
                            name=nc.get_next_instruction_name(), ins=[], outs=[])
                        nop.engine = inst.engine
                        nop.sync_info = mybir.SyncInfo(on_wait=list(ch), on_update=[])
                        new.append(nop)
                    inst.sync_info = mybir.SyncInfo(
                        on_wait=list(chunks[-1]), on_update=list(si.on_update))
                new.append(inst)
            bb.instructions = new


# weight/bias dram shapes (lhsT layout [K_in, M_out]; biases [p, cols])
_WSHAPES = [
    ("wn1", [HID_A, 2 * DU]), ("wn2", [2 * DU, DU]),
    ("we1", [HID_A, 2 * DU]), ("we2", [2 * DU, DU]),
    ("wm1", [3 * DU, 2 * DU]), ("wf", [6 * DU, 2 * DU]),
    ("wm2", [2 * DU, DS]),
    ("ws1", [DS, DS // 2]), ("ws2", [DS // 2, 1]),
    ("wa1", [3 * DS, MA]), ("wa1v", [80, MA]), ("wa2", [MA, HH]),
    ("wnev", [80, 2 * DU]),
    ("wsd1", [3 * DS, DS]), ("wsd2", [DS, DS]),
    ("wvd1", [3 * DV, DV]), ("wvd2", [DV, DV]),
    ("wg1", [DS + 3, DS]), ("wg2", [DS, 1]),
    ("wsc1", [3 * DS, DS]), ("wsc2", [DS, DS]),
    ("wvc1", [3 * DV, DV]), ("wvc2", [DV, DV]),
    ("bn1", [128, 2]), ("be1", [128, 2]),
    ("bm1", [128, 2]), ("bm2", [128, 1]),
    ("bs1", [64, 1]), ("bs2", [1, 1]),
    ("ba1", [128, 5]), ("ba2", [4, 1]),
    ("bsd1", [128, 1]), ("bsd2", [128, 1]),
    ("bvd1", [15, 1]), ("bvd2", [15, 1]),
    ("bg1", [128, 1]), ("bg2", [1, 1]),
    ("bsc1", [128, 1]), ("bsc2", [128, 1]),
    ("bvc1", [15, 1]), ("bvc2", [15, 1]),
    ("gnw", [128, 1]), ("gnb", [128, 1]), ("gnms", [128, 1]),
    ("lnw", [128, 1]), ("lnb", [128, 1]),
]
# loaded whole into one sbuf tile (partition dim <= 128)
_WSMALL = {"ws1", "ws2", "wsd2", "wvd2", "wg2", "wsc2",
           "wvc2", "bn1", "be1", "bm1", "bm2", "bs1", "bs2", "ba1", "ba2",
           "bsd1", "bsd2", "bvd1", "bvd2", "bg1", "bg2", "bsc1", "bsc2",
           "bvc1", "bvc2", "gnw", "gnb", "gnms", "lnw", "lnb"}



def _chunk_widths(S_t):
    """Split S_t (multiple of 128) into ceil(S_t/512) balanced widths,
    each a multiple of 128 and <= 512."""
    nch = math.ceil(S_t / 512)
    nsub = S_t // 128
    base = nsub // nch
    rem = nsub - base * nch
    return [(base + (1 if i < rem else 0)) * 128 for i in range(nch)]


def _build_nc(T, S_list, N_pad, G):
    assert G % 2 == 0, "graphs per core must be even (f32r even-N rule)"
    """Emit the per-core Bass program. T node tiles of 128, S_list[t] edge
    slots per node tile (mult of 128), N_pad = T*128 nodes, G graphs/core."""
    nc = bass.Bass()
    dp = nc.declare_dram_parameter
    S_list = list(S_list)
    S_off = [0]
    for s_ in S_list:
        S_off.append(S_off[-1] + s_)
    Stot = S_off[-1]
    S_max = max(S_list)
    KSUB = S_max // 128
    NCH = math.ceil(N_pad / 512)

    sss_e = dp("sss", [DS, 3 * Stot], FR, isOutput=False)  # si|sj|se per chunk
    vvv_e = dp("vvv", [80, Stot], FR, isOutput=False)      # vi@0, vj@32, ve@64
    oh_e = dp("oh", [Stot, 128], BF16, isOutput=False)     # [edge, node] one-hot
    ohT_e = dp("ohT", [T * 128, S_max], FR, isOutput=False)  # [node, edge] one-hot

    s_nd = dp("s_n", [DS, N_pad], FR, isOutput=False)
    v_nd = dp("v_n", [DV, N_pad], FR, isOutput=False)
    sclsb_nd = dp("sclsb_n", [DS, N_pad], FR, isOutput=False)
    vclsb_nd = dp("vclsb_n", [DV, N_pad], FR, isOutput=False)
    gh_oh = dp("gh_oh", [N_pad, 128], FR, isOutput=False)
    gh_ohT = dp("gh_ohT", [128, N_pad], FR, isOutput=False)
    invc_d = dp("invc", [128, 1], F32, isOutput=False)
    scls_d = dp("scls_g", [DS, G], FR, isOutput=False)
    vcls_d = dp("vcls_g", [DV, G], FR, isOutput=False)
    wd = {name: dp(name, shp, FR if name.startswith("w") else F32, isOutput=False)
          for name, shp in _WSHAPES}

    s_out_e = dp("s_out", [DS, N_pad], FR, isOutput=True)
    v_out_e = dp("v_out", [DV, N_pad], FR, isOutput=True)
    scls_out_e = dp("scls_out", [DS, G], F32, isOutput=True)
    vcls_out_e = dp("vcls_out", [DV, G], F32, isOutput=True)

    with nc.allow_low_precision(reason="float32r compute"), \
         tile.TileContext(nc) as tc, contextlib.ExitStack() as ctx:
        ep = ctx.enter_context  # shorthand
        cst = ep(tc.tile_pool(name="cst", bufs=1))
        per = ep(tc.tile_pool(name="per", bufs=1))
        ectx = contextlib.ExitStack()   # edge-stage pools, freed before node stage
        eep = ectx.enter_context
        epin = eep(tc.tile_pool(name="epin", bufs=3))
        epu = eep(tc.tile_pool(name="epu", bufs=5))
        eph = eep(tc.tile_pool(name="eph", bufs=3))
        uhp = eep(tc.tile_pool(name="uhp", bufs=9))
        epm = eep(tc.tile_pool(name="epm", bufs=3))
        epm1 = eep(tc.tile_pool(name="epm1", bufs=1))
        bufp = eep(tc.tile_pool(name="bufp", bufs=KSUB + 6))
        ohp = eep(tc.tile_pool(name="ohp", bufs=1))
        wmp = eep(tc.tile_pool(name="wmp", bufs=3))
        # PSUM: 8 banks total -> pb(3) + ptr(2) + pz(1) + pacc(2)
        pb = ep(tc.tile_pool(name="pb", bufs=4, space="PSUM"))
        ptr = ep(tc.tile_pool(name="ptr", bufs=2, space="PSUM"))
        pzp = ep(tc.tile_pool(name="pz", bufs=1, space="PSUM"))
        pacc = ep(tc.tile_pool(name="pacc", bufs=1, space="PSUM"))

        def PB(p_, f_):
            return pb.tile([p_, f_], F32, tag="pb", name="pbt")

        def PTR(p_, f_, dt_=F32):
            return ptr.tile([p_, f_], dt_, tag="ptr", name="ptrt")

        # ---- constants ----
        ident_f = cst.tile([128, 128], F32)
        make_identity(nc, ident_f[:])
        ident = cst.tile([128, 128], FR)
        nc.vector.tensor_copy(ident[:], ident_f[:])
        cscr = cst.tile([128, 1], F32)
        nc.vector.memset(cscr[:], 1.0)
        ones15 = cst.tile([15, 1], FR)
        nc.vector.tensor_copy(ones15[:], cscr[0:15, :])
        cscr1x = cst.tile([1, 128], F32)
        nc.vector.memset(cscr1x[:], 1.0)
        ones1x = cst.tile([1, 128], FR)
        nc.vector.tensor_copy(ones1x[:], cscr1x[:])
        cscr2 = cst.tile([128, 1], F32)
        nc.vector.memset(cscr2[:], 1.0 / DS)
        oo128 = cst.tile([128, 1], FR)
        nc.vector.tensor_copy(oo128[:], cscr2[:])
        eps5 = cst.tile([128, 1], F32)
        nc.vector.memset(eps5[:], 1e-5)

        W = {}
        for name in _WSMALL:
            W[name] = cst.tile(list(wd[name].shape), FR if name.startswith("w") else F32,
                               tag=f"w_{name}", name=f"w_{name}")
            nc.sync.dma_start(out=W[name][:], in_=wd[name][:])

        def ksplit(name, splits):
            """Load dram weight rows into separate tiles per K-chunk."""
            tiles, r0 = [], 0
            for kk in splits:
                t_ = cst.tile([kk, wd[name].shape[1]], FR, tag=f"{name}_{r0}", name=f"{name}_{r0}")
                nc.sync.dma_start(out=t_[:], in_=wd[name][r0:r0 + kk, :])
                tiles.append(t_)
                r0 += kk
            return tiles

        wn1_k = ksplit("wn1", [128])
        we1_k = ksplit("we1", [128])
        wa1v = cst.tile([80, MA], FR)
        nc.sync.dma_start(out=wa1v[:], in_=wd["wa1v"][:])
        wnev = cst.tile([80, 2 * DU], FR)
        nc.sync.dma_start(out=wnev[:], in_=wd["wnev"][:])
        wf_k = ksplit("wf", [128] * 6)
        wm2_k = ksplit("wm2", [128, 128])
        wa1_k = ksplit("wa1", [128, 128, 128])  # si,sj,se rows
        wa2_k = ksplit("wa2", [128, 128, 128, 128, 60])
        wsd1_k = ksplit("wsd1", [128, 128, 128])
        wsc1_k = ksplit("wsc1", [128, 128, 128])
        wvd1_k = ksplit("wvd1", [15, 15, 15])
        wvc1_k = ksplit("wvc1", [15, 15, 15])
        wg1_k = ksplit("wg1", [1, 1, 1, 128])

        invc = cst.tile([128, 1], F32)
        nc.sync.dma_start(out=invc[:], in_=invc_d[:])
        scls_s = cst.tile([DS, G], FR)
        nc.sync.dma_start(out=scls_s[:], in_=scls_d[:])
        vcls_s = cst.tile([DV, G], FR)
        nc.sync.dma_start(out=vcls_s[:], in_=vcls_d[:])

        # ---- persistent buffers ----
        agg_sb = [per.tile([128, 144], FR, tag=f"agg{t}", name=f"agg{t}") for t in range(T)]
        sagg_fn = per.tile([DS, N_pad], FR)
        vagg_fn = per.tile([DV, N_pad], FR)
        x_fn = per.tile([DS, N_pad], FR)       # x, later overwritten by xc
        sout_fn = per.tile([DS, N_pad], FR)
        vout_fn = per.tile([16, N_pad], FR)
        mean_gf = per.tile([128, 128], FR)
        nm_g = per.tile([128, 144], FR)
        em_g = per.tile([128, 144], FR)

        # ================= EDGE STAGE =================
        for t in range(T):
            S_t = S_list[t]
            KSUB_t = S_t // 128
            oh_t = ohp.tile([128, KSUB, 128], BF16, tag="oh", name="oh_t")
            nc.sync.dma_start(
                out=oh_t[:, 0:KSUB_t, :],
                in_=oh_e[S_off[t]:S_off[t + 1], :].rearrange(
                    "(k p) n -> p k n", p=128))
            ohT_t = ohp.tile([128, S_max], FR, tag="ohT", name="ohT_t")
            nc.sync.dma_start(out=ohT_t[:, 0:S_t],
                              in_=ohT_e[t * 128:(t + 1) * 128, 0:S_t])

            psz = pzp.tile([128, 4], F32, tag="pz")
            bufs_t = []
            widths = _chunk_widths(S_t)
            w_off = [0]
            for w_ in widths:
                w_off.append(w_off[-1] + w_)
            for ci, cw in enumerate(widths):
                e0 = S_off[t] + w_off[ci]
                sss_c = epin.tile([DS, 3 * cw], FR, tag="sss")
                nc.sync.dma_start(out=sss_c[:], in_=sss_e[:, 3 * e0:3 * e0 + 3 * cw])
                vvv_c = epin.tile([80, cw], FR, tag="vvv")
                nc.sync.dma_start(out=vvv_c[:], in_=vvv_e[:, e0:e0 + cw])
                si_c, sj_c, se_c = (sss_c[:, 0:cw], sss_c[:, cw:2 * cw],
                                    sss_c[:, 2 * cw:3 * cw])
                vi_c, vj_c, ve_c = (vvv_c[0:15, :], vvv_c[32:47, :],
                                    vvv_c[64:79, :])

                # interleaved emission: attn m-chunks woven between u-MLP
                # stages so PE always has independent matmuls to run while
                # ACT processes silus.
                def u_hid(s_in, v_in, v_w1, w1k, b1, m):
                    ph = PB(128, cw)
                    nc.tensor.matmul(ph[:], _r(w1k[0][:, 128 * m:128 * (m + 1)]),
                                     _r(s_in), start=True, stop=False)
                    nc.tensor.matmul(ph[:], _r(v_w1[:, 128 * m:128 * (m + 1)]),
                                     _r(v_in), start=False, stop=True)
                    h = uhp.tile([128, cw], FR, tag="uh", name="h")
                    nc.scalar.activation(h[:], ph[:], AF.Silu, bias=b1[:, m:m + 1])
                    return h

                praw = PB(4, cw)

                def attn_m(m):
                    mw = min(128, MA - 128 * m)
                    ph = PB(128, cw)
                    ins = [(wa1_k[0][:, 128 * m:128 * m + mw], si_c),
                           (wa1_k[1][:, 128 * m:128 * m + mw], sj_c),
                           (wa1_k[2][:, 128 * m:128 * m + mw], se_c),
                           (wa1v[:, 128 * m:128 * m + mw], vvv_c[:])]
                    for ki, (wt, xin) in enumerate(ins):
                        nc.tensor.matmul(ph[:mw, :], _r(wt), _r(xin),
                                         start=(ki == 0), stop=(ki == 3))
                    h = eph.tile([128, cw], FR, tag="ah", name="ah")
                    nc.scalar.activation(h[:mw, :], ph[:mw, :], AF.Silu,
                                         bias=W["ba1"][:mw, m:m + 1])
                    nc.tensor.matmul(praw[:], _r(wa2_k[m][:]), _r(h[:mw, :]),
                                     start=(m == 0), stop=(m == 4))

                attn_m(0)
                ui0 = u_hid(si_c, vi_c, wnev[0:15, :], wn1_k, W["bn1"], 0)
                ui1 = u_hid(si_c, vi_c, wnev[0:15, :], wn1_k, W["bn1"], 1)
                attn_m(1)
                uj0 = u_hid(sj_c, vj_c, wnev[32:47, :], wn1_k, W["bn1"], 0)
                uj1 = u_hid(sj_c, vj_c, wnev[32:47, :], wn1_k, W["bn1"], 1)
                attn_m(2)
                ue0 = u_hid(se_c, ve_c, wnev[64:79, :], we1_k, W["be1"], 0)
                ue1 = u_hid(se_c, ve_c, wnev[64:79, :], we1_k, W["be1"], 1)
                attn_m(3)
                uhids = [ui0, ui1, uj0, uj1, ue0, ue1]
                attn_m(4)

                # msg L1 folded over u-hiddens: Wf = W2_{node,edge} @ W1_msg chunks
                mh = []
                for m in range(2):
                    ph = PB(128, cw)
                    for ki, hh in enumerate(uhids):
                        nc.tensor.matmul(ph[:], _r(wf_k[ki][:, 128 * m:128 * (m + 1)]),
                                         _r(hh[:]), start=(ki == 0), stop=(ki == 5))
                    h = uhp.tile([128, cw], FR, tag="uh", name="mhh")
                    nc.scalar.activation(h[:], ph[:], AF.Silu, bias=W["bm1"][:, m:m + 1])
                    mh.append(h)
                pm = PB(128, cw)
                nc.tensor.matmul(pm[:], _r(wm2_k[0][:]), _r(mh[0][:]),
                                 start=True, stop=False)
                nc.tensor.matmul(pm[:], _r(wm2_k[1][:]), _r(mh[1][:]),
                                 start=False, stop=True)
                msg = epm.tile([128, cw], FR, tag="msg")
                nc.vector.tensor_scalar_add(msg[:], pm[:], W["bm2"][:, 0:1])

                # coeff = mlp2(msg) 128->64->1
                pch = PB(64, cw)
                nc.tensor.matmul(pch[:], _r(W["ws1"][:]), _r(msg[:]),
                                 start=True, stop=True)
                chh = eph.tile([64, cw], FR, tag="ch")
                nc.scalar.activation(chh[:], pch[:], AF.Silu, bias=W["bs1"][:, 0:1])
                pco = PB(1, cw)
                nc.tensor.matmul(pco[:], _r(W["ws2"][:]), _r(chh[:]),
                                 start=True, stop=True)
                coeff = epm1.tile([1, cw], FR, tag="cf")
                nc.vector.tensor_scalar_add(coeff[:], pco[:], W["bs2"][:, 0:1])

                # v_msg = v_edge * coeff (coeff broadcast to 15 partitions);
                # er lands in rows 32:36 of the same tile so one transpose
                # moves both to [edge_p, feat].
                pcb = PB(15, cw)
                nc.tensor.matmul(pcb[:], _r(ones1x[0:1, 0:15]), _r(coeff[:]),
                                 start=True, stop=True)
                vmsg_c = epm1.tile([36, cw], FR, tag="vmsgc")
                nc.vector.tensor_mul(vmsg_c[0:15, :], ve_c, pcb[:])
                nc.scalar.activation(vmsg_c[32:36, :], praw[:], AF.Exp,
                                     bias=W["ba2"][:, 0:1])

                # transpose msg/(vmsg|er) to [edge_p, feat]; z aggregation
                for kk in range(cw // 128):
                    kg = w_off[ci] // 128 + kk
                    sl = slice(128 * kk, 128 * (kk + 1))
                    buf = bufp.tile([128, 164], BF16, tag="buf")
                    bufs_t.append(buf)
                    ptm = PTR(128, 128, FR)
                    nc.tensor.transpose(ptm[:], msg[:, sl], ident[:])
                    nc.vector.tensor_copy(buf[:, 0:128], ptm[:])
                    ptv = PTR(128, 36, FR)
                    nc.tensor.transpose(ptv[:], vmsg_c[:, sl], ident[0:36, 0:36])
                    nc.vector.tensor_copy(buf[:, 128:164], ptv[:])
                    nc.tensor.matmul(psz[:], _r(oh_t[:, kg, :]), _r(buf[:, 160:164]),
                                     start=(kg == 0), stop=(kg == KSUB_t - 1))

            # z -> 1/max(z, eps)
            rz = epm.tile([128, 4], FR, tag="rz")
            nc.vector.tensor_scalar(out=rz[:], in0=psz[:], scalar1=1e-30,
                                    scalar2=4.0, op0=OP.max, op1=OP.mult)
            nc.vector.reciprocal(rz[:], rz[:])

            # pass B: broadcast 1/z to edges, weight msgs, aggregate
            pag = pacc.tile([128, 144], F32, tag="pacc")
            for kg in range(KSUB_t):
                buf = bufs_t[kg]
                pzb = PTR(128, 4)
                nc.tensor.matmul(pzb[:], _r(ohT_t[:, 128 * kg:128 * (kg + 1)]),
                                 _r(rz[:]), start=True, stop=True)
                scr = wmp.tile([128, 4], F32, tag="scr")
                wT = wmp.tile([128, 1], F32, tag="wT")
                nc.vector.tensor_mul(scr[:], buf[:, 160:164], pzb[:])
                nc.vector.reduce_sum(wT[:], scr[:], axis=mybir.AxisListType.X)
                wmsg = wmp.tile([128, 144], BF16, tag="wmsg")
                nc.vector.tensor_scalar_mul(wmsg[:], buf[:, 0:144], wT[:, 0:1])
                nc.tensor.matmul(pag[:], _r(oh_t[:, kg, :]), _r(wmsg[:]),
                                 start=(kg == 0), stop=(kg == KSUB_t - 1))
            nc.vector.tensor_copy(agg_sb[t][:], pag[:])
            pta = PTR(128, 128, FR)
            nc.tensor.transpose(pta[:], agg_sb[t][:, 0:128], ident[:])
            nc.vector.tensor_copy(sagg_fn[:, 128 * t:128 * (t + 1)], pta[:])
            ptb = PTR(16, 128, FR)
            nc.tensor.transpose(ptb[:], agg_sb[t][:, 128:144], ident[:])
            nc.vector.tensor_copy(vagg_fn[:, 128 * t:128 * (t + 1)], ptb[0:15, :])

        # ================= NODE STAGE =================
        ectx.close()
        nsp = ep(tc.tile_pool(name="nsp", bufs=2))
        nsp1 = ep(tc.tile_pool(name="nsp1", bufs=1))
        gp = ep(tc.tile_pool(name="gp", bufs=1))
        ghoh = gp.tile([128, T * 128], FR)
        for t in range(T):
            nc.sync.dma_start(out=ghoh[:, 128 * t:128 * (t + 1)],
                              in_=gh_oh[128 * t:128 * (t + 1), :])
        ghohT = gp.tile([128, N_pad], FR)
        nc.sync.dma_start(out=ghohT[:], in_=gh_ohT[:])
        for ci in range(NCH):
            n0 = ci * 512
            cw = min(512, N_pad - n0)
            sl = slice(n0, n0 + cw)
            s_c = nsp.tile([DS, cw], FR, tag="s_c")
            nc.sync.dma_start(out=s_c[:], in_=s_nd[:, sl])
            scb_c = nsp.tile([DS, cw], FR, tag="scb")
            nc.sync.dma_start(out=scb_c[:], in_=sclsb_nd[:, sl])
            v_c = nsp1.tile([DV, cw], FR, tag="v_c")
            nc.sync.dma_start(out=v_c[:], in_=v_nd[:, sl])
            vcb_c = nsp1.tile([DV, cw], FR, tag="vcb")
            nc.sync.dma_start(out=vcb_c[:], in_=vclsb_nd[:, sl])

            # s_delta MLP 384->128->128 ; x = s + s_delta
            ph = PB(128, cw)
            for ki, xin in enumerate([s_c[:], sagg_fn[:, sl], scb_c[:]]):
                nc.tensor.matmul(ph[:], _r(wsd1_k[ki][:]), _r(xin),
                                 start=(ki == 0), stop=(ki == 2))
            h = nsp.tile([128, cw], FR, tag="nh")
            nc.scalar.activation(h[:], ph[:], AF.Silu, bias=W["bsd1"][:, 0:1])
            po = PB(128, cw)
            nc.tensor.matmul(po[:], _r(W["wsd2"][:]), _r(h[:]), start=True, stop=True)
            nc.vector.scalar_tensor_tensor(
                out=x_fn[:, sl], in0=po[:], scalar=W["bsd2"][:, 0:1],
                in1=s_c[:], op0=OP.add, op1=OP.add)

            # v_delta MLP 45->15->15
            pvh = PB(15, cw)
            for ki, xin in enumerate([v_c[:], vagg_fn[:, sl], vcb_c[:]]):
                nc.tensor.matmul(pvh[:], _r(wvd1_k[ki][:]), _r(xin),
                                 start=(ki == 0), stop=(ki == 2))
            vh = nsp1.tile([15, cw], FR, tag="vh")
            nc.scalar.activation(vh[:], pvh[:], AF.Silu, bias=W["bvd1"][:, 0:1])
            pvo = PB(15, cw)
            nc.tensor.matmul(pvo[:], _r(W["wvd2"][:]), _r(vh[:]), start=True, stop=True)
            vd_c = nsp1.tile([DV, cw], FR, tag="vd_c")
            nc.vector.tensor_scalar_add(vd_c[:], pvo[:], W["bvd2"][:, 0:1])

            # gate: concat(|v|, |vd|, cos, s) -> 128 -> 1 -> sigmoid
            t2 = nsp1.tile([15, cw], FR, tag="t2")
            nc.vector.tensor_mul(t2[:], v_c[:], v_c[:])
            pvn = PB(1, cw)
            nc.tensor.matmul(pvn[:], _r(ones15[:]), _r(t2[:]), start=True, stop=True)
            vn = nsp1.tile([1, cw], FR, tag="vn")
            nc.scalar.activation(vn[:], pvn[:], AF.Sqrt)
            t3 = nsp1.tile([15, cw], FR, tag="t3")
            nc.vector.tensor_mul(t3[:], vd_c[:], vd_c[:])
            pdn = PB(1, cw)
            nc.tensor.matmul(pdn[:], _r(ones15[:]), _r(t3[:]), start=True, stop=True)
            dn = nsp1.tile([1, cw], FR, tag="dn")
            nc.scalar.activation(dn[:], pdn[:], AF.Sqrt)
            t4 = nsp1.tile([15, cw], FR, tag="t4")
            nc.vector.tensor_mul(t4[:], v_c[:], vd_c[:])
            pdo = PB(1, cw)
            nc.tensor.matmul(pdo[:], _r(ones15[:]), _r(t4[:]), start=True, stop=True)
            den = nsp1.tile([1, cw], F32, tag="den")
            nc.vector.tensor_mul(den[:], vn[:], dn[:])
            nc.vector.tensor_scalar_add(den[:], den[:], 1e-6)
            nc.vector.reciprocal(den[:], den[:])
            cosn = nsp1.tile([1, cw], FR, tag="cosn")
            nc.vector.tensor_mul(cosn[:], pdo[:], den[:])
            pgh = PB(128, cw)
            nc.tensor.matmul(pgh[:], _r(wg1_k[0][:]), _r(vn[:]), start=True, stop=False)
            nc.tensor.matmul(pgh[:], _r(wg1_k[1][:]), _r(dn[:]), start=False, stop=False)
            nc.tensor.matmul(pgh[:], _r(wg1_k[2][:]), _r(cosn[:]), start=False, stop=False)
            nc.tensor.matmul(pgh[:], _r(wg1_k[3][:]), _r(s_c[:]),
                             start=False, stop=True)
            gh = nsp.tile([128, cw], FR, tag="nh")
            nc.scalar.activation(gh[:], pgh[:], AF.Silu, bias=W["bg1"][:, 0:1])
            pgo = PB(1, cw)
            nc.tensor.matmul(pgo[:], _r(W["wg2"][:]), _r(gh[:]), start=True, stop=True)
            gate = nsp1.tile([1, cw], FR, tag="gate")
            nc.scalar.activation(gate[:], pgo[:], AF.Sigmoid, bias=W["bg2"][:, 0:1])
            pgb = PB(15, cw)
            nc.tensor.matmul(pgb[:], _r(ones1x[0:1, 0:15]), _r(gate[:]),
                             start=True, stop=True)
            nc.vector.tensor_mul(vout_fn[0:15, sl], vd_c[:], pgb[:])
            nc.vector.tensor_add(vout_fn[0:15, sl], vout_fn[0:15, sl], v_c[:])

        # ---- GraphNorm (one-pass stats: aggregate [x | x^2] together) ----
        pgs = pacc.tile([128, 256], F32, tag="pacc", name="pgs")
        for t in range(T):
            sq = nsp.tile([128, 128], FR, tag="sq")
            nc.vector.tensor_mul(sq[:], x_fn[:, 128 * t:128 * (t + 1)],
                                 x_fn[:, 128 * t:128 * (t + 1)])
            ptx = PTR(128, 128, FR)
            nc.tensor.transpose(ptx[:], x_fn[:, 128 * t:128 * (t + 1)], ident[:])
            ptq = PTR(128, 128, FR)
            nc.tensor.transpose(ptq[:], sq[:], ident[:])
            xts = nsp.tile([128, 256], FR, tag="xts")
            nc.vector.tensor_copy(xts[:, 0:128], ptx[:])
            nc.vector.tensor_copy(xts[:, 128:256], ptq[:])
            nc.tensor.matmul(pgs[:], _r(ghoh[:, 128 * t:128 * (t + 1)]), _r(xts[:]),
                             start=(t == 0), stop=(t == T - 1))
        nc.vector.tensor_scalar_mul(mean_gf[:], pgs[:, 0:128], invc[:, 0:1])
        ex2_gf = per.tile([128, 128], FR)
        nc.vector.tensor_scalar_mul(ex2_gf[:], pgs[:, 128:256], invc[:, 0:1])
        for ci in range(NCH):
            n0 = ci * 512
            cw = min(512, N_pad - n0)
            sl = slice(n0, n0 + cw)
            pmb = PB(128, cw)
            nc.tensor.matmul(pmb[:], _r(mean_gf[:]), _r(ghohT[:, sl]),
                             start=True, stop=True)
            pxb = PB(128, cw)
            nc.tensor.matmul(pxb[:], _r(ex2_gf[:]), _r(ghohT[:, sl]),
                             start=True, stop=True)
            tmb = nsp.tile([128, cw], F32, tag="nh")      # gms * mean_b
            nc.vector.tensor_scalar_mul(tmb[:], pmb[:], W["gnms"][:, 0:1])
            qb = nsp.tile([128, cw], F32, tag="qb")       # 2*mean_b - tmb
            nc.vector.scalar_tensor_tensor(out=qb[:], in0=pmb[:], scalar=2.0,
                                           in1=tmb[:], op0=OP.mult, op1=OP.subtract)
            vb = nsp.tile([128, cw], F32, tag="vb")       # (2g-g^2) mean^2
            nc.vector.tensor_mul(vb[:], tmb[:], qb[:])
            nc.vector.tensor_sub(x_fn[:, sl], x_fn[:, sl], tmb[:])   # xc in place
            varb = nsp.tile([128, cw], F32, tag="vrb")
            nc.vector.tensor_tensor(out=varb[:], in0=pxb[:], in1=vb[:],
                                    op=OP.subtract)
            nc.scalar.activation(varb[:], varb[:], AF.Sqrt, bias=eps5[:, 0:1])
            nc.vector.reciprocal(varb[:], varb[:])
            nc.vector.tensor_mul(sout_fn[:, sl], x_fn[:, sl], varb[:])
            nc.vector.tensor_scalar(out=sout_fn[:, sl], in0=sout_fn[:, sl],
                                    scalar1=W["gnw"][:, 0:1], scalar2=W["gnb"][:, 0:1],
                                    op0=OP.mult, op1=OP.add)
            nc.sync.dma_start(out=s_out_e[:, sl], in_=sout_fn[:, sl])
            nc.sync.dma_start(out=v_out_e[:, sl], in_=vout_fn[0:15, sl])

        # ---- CLS stage ----
        pnm = pacc.tile([128, 144], F32, tag="pacc")
        pem = pacc.tile([128, 144], F32, tag="pacc")
        for t in range(T):
            so_t = nsp.tile([128, 144], FR, tag="sot")
            ptx = PTR(128, 128, FR)
            nc.tensor.transpose(ptx[:], sout_fn[:, 128 * t:128 * (t + 1)], ident[:])
            nc.vector.tensor_copy(so_t[:, 0:128], ptx[:])
            ptv = PTR(128, 16, FR)
            nc.tensor.transpose(ptv[:], vout_fn[:, 128 * t:128 * (t + 1)],
                                ident[0:16, 0:16])
            nc.vector.tensor_copy(so_t[:, 128:144], ptv[:])
            nc.tensor.matmul(pnm[:], _r(ghoh[:, 128 * t:128 * (t + 1)]), _r(so_t[:]),
                             start=(t == 0), stop=(t == T - 1))
            nc.tensor.matmul(pem[:], _r(ghoh[:, 128 * t:128 * (t + 1)]),
                             _r(agg_sb[t][:]), start=(t == 0), stop=(t == T - 1))
        nc.vector.tensor_scalar_mul(nm_g[:], pnm[:], invc[:, 0:1])
        nc.vector.tensor_scalar_mul(em_g[:], pem[:], invc[:, 0:1])

        def tr_fg(src_ap, P, Fw, tag):
            pt = PTR(Fw, P, FR)
            nc.tensor.transpose(pt[:], src_ap, ident[0:P, 0:P])
            dst = nsp.tile([Fw, P], FR, tag=tag)
            nc.vector.tensor_copy(dst[:], pt[:])
            return dst

        snm_fg = tr_fg(nm_g[0:G, 0:128], G, 128, "snm")
        sem_fg = tr_fg(em_g[0:G, 0:128], G, 128, "sem")
        vnm_fg = tr_fg(nm_g[0:G, 128:144], G, 16, "vnm")
        vem_fg = tr_fg(em_g[0:G, 128:144], G, 16, "vem")

        # s_cls update + layernorm over features
        ph = PB(128, G)
        for ki, xin in enumerate([scls_s[:], snm_fg[:], sem_fg[:]]):
            nc.tensor.matmul(ph[:], _r(wsc1_k[ki][:]), _r(xin),
                             start=(ki == 0), stop=(ki == 2))
        h = nsp.tile([128, G], FR, tag="nh")
        nc.scalar.activation(h[:], ph[:], AF.Silu, bias=W["bsc1"][:, 0:1])
        po = PB(128, G)
        nc.tensor.matmul(po[:], _r(W["wsc2"][:]), _r(h[:]), start=True, stop=True)
        yc = nsp.tile([128, G], FR, tag="yc")
        nc.vector.scalar_tensor_tensor(out=yc[:], in0=po[:], scalar=W["bsc2"][:, 0:1],
                                       in1=scls_s[:], op0=OP.add, op1=OP.add)
        pmu = PB(1, G)
        nc.tensor.matmul(pmu[:], _r(oo128[:]), _r(yc[:]), start=True, stop=True)
        mu = nsp.tile([1, G], FR, tag="mu")
        nc.vector.tensor_copy(mu[:], pmu[:])
        pmb = PB(128, G)
        nc.tensor.matmul(pmb[:], _r(ones1x[0:1, :]), _r(mu[:]), start=True, stop=True)
        ycc = nsp.tile([128, G], FR, tag="ycc")
        nc.vector.tensor_sub(ycc[:], yc[:], pmb[:])
        sqg = nsp.tile([128, G], FR, tag="sqg")
        nc.vector.tensor_mul(sqg[:], ycc[:], ycc[:])
        pvv = PB(1, G)
        nc.tensor.matmul(pvv[:], _r(oo128[:]), _r(sqg[:]), start=True, stop=True)
        rs = nsp.tile([1, G], FR, tag="rs")
        nc.scalar.activation(rs[:], pvv[:], AF.Sqrt, bias=eps5[0:1, 0:1])
        nc.vector.reciprocal(rs[:], rs[:])
        prb = PB(128, G)
        nc.tensor.matmul(prb[:], _r(ones1x[0:1, :]), _r(rs[:]), start=True, stop=True)
        sco = nsp.tile([128, G], F32, tag="sco")
        nc.vector.tensor_mul(sco[:], ycc[:], prb[:])
        nc.vector.tensor_scalar(out=sco[:], in0=sco[:], scalar1=W["lnw"][:, 0:1],
                                scalar2=W["lnb"][:, 0:1], op0=OP.mult, op1=OP.add)
        nc.sync.dma_start(out=scls_out_e[:], in_=sco[:])

        # v_cls update
        pvh = PB(15, G)
        for ki, xin in enumerate([vcls_s[:], vnm_fg[0:15, :], vem_fg[0:15, :]]):
            nc.tensor.matmul(pvh[:], _r(wvc1_k[ki][:]), _r(xin),
                             start=(ki == 0), stop=(ki == 2))
        vh = nsp.tile([15, G], FR, tag="vh")
        nc.scalar.activation(vh[:], pvh[:], AF.Silu, bias=W["bvc1"][:, 0:1])
        pvo = PB(15, G)
        nc.tensor.matmul(pvo[:], _r(W["wvc2"][:]), _r(vh[:]), start=True, stop=True)
        vco = nsp.tile([15, G], F32, tag="vco")
        nc.vector.scalar_tensor_tensor(out=vco[:], in0=pvo[:], scalar=W["bvc2"][:, 0:1],
                                       in1=vcls_s[:], op0=OP.add, op1=OP.add)
        nc.sync.dma_start(out=vcls_out_e[:], in_=vco[:])

    _split_waits(nc)
    return nc


def _prep(s, v, s_edge, v_edge, edge_index, s_cls, v_cls, batch, params):
    """Host-side index prep + sharding. Returns (meta, in_maps)."""
    s = np.asarray(s, np.float32)
    v = np.asarray(v, np.float32)
    s_edge = np.asarray(s_edge, np.float32)
    v_edge = np.asarray(v_edge, np.float32)
    s_cls = np.asarray(s_cls, np.float32)
    v_cls = np.asarray(v_cls, np.float32)
    ei = np.asarray(edge_index).astype(np.int64)
    batch_np = np.asarray(batch).astype(np.int64)
    row, col = ei[0], ei[1]
    N, E, B = s.shape[0], row.shape[0], s_cls.shape[0]
    G = B // C

    g_lo = np.arange(C, dtype=np.int64) * G
    n_lo = np.searchsorted(batch_np, g_lo).astype(np.int64)
    n_hi = np.append(n_lo[1:], N).astype(np.int64)
    n_cnt = n_hi - n_lo
    N_pad = 128 * math.ceil(n_cnt.max() / 128)
    T = N_pad // 128

    edge_core = np.searchsorted(n_hi, row, side="right")
    local_row = row - n_lo[edge_core]
    tile_of_edge = local_row // 128
    key = edge_core * T + tile_of_edge
    counts = np.bincount(key, minlength=C * T).reshape(C, T)
    S_list = np.maximum(512, 128 * np.ceil(counts.max(0) / 128).astype(np.int64))
    S_off = np.zeros(T + 1, np.int64)
    S_off[1:] = np.cumsum(S_list)
    Stot1 = int(S_off[-1])
    S_max = int(S_list.max())
    order = np.argsort(key, kind="stable")
    key_o = key[order]
    starts = np.zeros(C * T + 1, np.int64)
    starts[1:] = np.cumsum(counts.reshape(-1))
    rank = np.arange(E, dtype=np.int64) - starts[key_o]
    core_o = key_o // T
    tile_o = key_o % T
    gslot = core_o * Stot1 + S_off[tile_o] + rank
    row_o, col_o = row[order], col[order]
    lr_o = local_row[order]

    CTS = C * Stot1
    si_all = np.zeros((CTS, DS), np.float32)
    sj_all = np.zeros((CTS, DS), np.float32)
    se_all = np.zeros((CTS, DS), np.float32)
    vi_all = np.zeros((CTS, DV), np.float32)
    vj_all = np.zeros((CTS, DV), np.float32)
    ve_all = np.zeros((CTS, DV), np.float32)
    chunk_bounds = []
    for t in range(T):
        St = int(S_list[t])
        off = int(S_off[t])
        for w_ in _chunk_widths(St):
            chunk_bounds.append((off, off + w_))
            off += w_
    si_all[gslot] = s[row_o]
    sj_all[gslot] = s[col_o]
    se_all[gslot] = s_edge[order]
    vi_all[gslot] = v[row_o]
    vj_all[gslot] = v[col_o]
    ve_all[gslot] = v_edge[order]
    oh_all = np.zeros((CTS, 128), np.float32)
    oh_all[gslot, lr_o % 128] = 1.0
    ohT_all = np.zeros((C * T * 128, S_max), np.float32)
    ohT_all[key_o * 128 + lr_o % 128, rank] = 1.0

    cnt = np.bincount(batch_np, minlength=B).astype(np.float32)
    inv_cnt = 1.0 / np.maximum(cnt, 1.0)

    p = params
    wn1, bn1, wn2, bn2 = [np.asarray(a, np.float32) for a in p["node"]]
    we1, be1, we2, be2 = [np.asarray(a, np.float32) for a in p["edge"]]
    wm1, bm1, wm2, bm2 = [np.asarray(a, np.float32) for a in p["msg"]]
    ws1, bs1, ws2, bs2 = [np.asarray(a, np.float32) for a in p["s2v"]]
    wsd1, bsd1, wsd2, bsd2 = [np.asarray(a, np.float32) for a in p["s_delta"]]
    wvd1, bvd1, wvd2, bvd2 = [np.asarray(a, np.float32) for a in p["v_delta"]]
    wg1, bg1, wg2, bg2 = [np.asarray(a, np.float32) for a in p["gate"]]
    wsc1, bsc1, wsc2, bsc2 = [np.asarray(a, np.float32) for a in p["s_cls"]]
    wvc1, bvc1, wvc2, bvc2 = [np.asarray(a, np.float32) for a in p["v_cls"]]
    aw1 = np.asarray(p["attn_W1"], np.float32)
    ab1 = np.asarray(p["attn_b1"], np.float32)
    aw2 = np.asarray(p["attn_W2"], np.float32)
    ab2 = np.asarray(p["attn_b2"], np.float32)
    Fdim = aw1.shape[1]
    wa1f = np.ascontiguousarray(np.transpose(aw1, (1, 0, 2)).reshape(Fdim, MA))
    wa1 = wa1f[0:384]
    wa1v = np.zeros((80, MA), np.float32)
    wa1v[0:15] = wa1f[384:399]
    wa1v[32:47] = wa1f[399:414]
    wa1v[64:79] = wa1f[414:429]
    wnev = np.zeros((80, 2 * DU), np.float32)
    wnev[0:15] = wn1[128:143]
    wnev[32:47] = wn1[128:143]
    wnev[64:79] = we1[128:143]
    wa2 = np.zeros((MA, HH), np.float32)
    for hh in range(HH):
        wa2[hh * HID_A:(hh + 1) * HID_A, hh] = aw2[hh]
    ba1 = ab1.reshape(MA)
    ba1_pad = np.zeros((128, 5), np.float32)
    for m in range(5):
        mw = min(128, MA - 128 * m)
        ba1_pad[:mw, m] = ba1[128 * m:128 * m + mw]
    bm1_eff = bm1 + bn2 @ wm1[0:DU] + bn2 @ wm1[DU:2 * DU] + be2 @ wm1[2 * DU:3 * DU]
    wf = np.concatenate([wn2 @ wm1[0:DU], wn2 @ wm1[DU:2 * DU],
                         we2 @ wm1[2 * DU:3 * DU]], axis=0)  # [768, 256]

    weights = {
        "wn1": wn1, "wn2": wn2, "we1": we1, "we2": we2,
        "wm1": wm1, "wf": wf, "wm2": wm2, "ws1": ws1, "ws2": ws2,
        "wa1": wa1, "wa1v": wa1v, "wnev": wnev, "wa2": wa2,
        "wsd1": wsd1, "wsd2": wsd2,
        "wvd1": wvd1, "wvd2": wvd2, "wg1": wg1, "wg2": wg2,
        "wsc1": wsc1, "wsc2": wsc2, "wvc1": wvc1, "wvc2": wvc2,
        "bn1": np.ascontiguousarray(np.stack([bn1[:128], bn1[128:]], 1)),
        "be1": np.ascontiguousarray(np.stack([be1[:128], be1[128:]], 1)),
        "bm1": np.ascontiguousarray(np.stack([bm1_eff[:128], bm1_eff[128:]], 1)),
        "bm2": bm2[:, None], "bs1": bs1[:, None], "bs2": bs2[:, None],
        "ba1": ba1_pad, "ba2": ab2[:, None],
        "bsd1": bsd1[:, None], "bsd2": bsd2[:, None],
        "bvd1": bvd1[:, None], "bvd2": bvd2[:, None],
        "bg1": bg1[:, None], "bg2": bg2[:, None],
        "bsc1": bsc1[:, None], "bsc2": bsc2[:, None],
        "bvc1": bvc1[:, None], "bvc2": bvc2[:, None],
        "gnw": np.asarray(p["gn_weight"], np.float32)[:, None],
        "gnb": np.asarray(p["gn_bias"], np.float32)[:, None],
        "gnms": np.asarray(p["gn_mean_scale"], np.float32)[:, None],
        "lnw": np.asarray(p["ln_w"], np.float32)[:, None],
        "lnb": np.asarray(p["ln_b"], np.float32)[:, None],
    }
    weights = {k: np.ascontiguousarray(a) for k, a in weights.items()}

    in_maps = []
    for c in range(C):
        lo, hi, ncn = int(n_lo[c]), int(n_hi[c]), int(n_cnt[c])
        esl = slice(c * Stot1, (c + 1) * Stot1)
        bloc = batch_np[lo:hi] - c * G
        siT = si_all[esl].T
        sjT = sj_all[esl].T
        seT = se_all[esl].T
        cols = []
        for (a, b) in chunk_bounds:
            cols += [siT[:, a:b], sjT[:, a:b], seT[:, a:b]]
        sss = np.ascontiguousarray(np.concatenate(cols, axis=1))
        vvv = np.zeros((80, Stot1), np.float32)
        vvv[0:15] = vi_all[esl].T
        vvv[32:47] = vj_all[esl].T
        vvv[64:79] = ve_all[esl].T
        m = {
            "sss": sss,
            "vvv": vvv,
            "oh": np.ascontiguousarray(oh_all[esl]).astype(_bf16),
            "ohT": np.ascontiguousarray(ohT_all[c * T * 128:(c + 1) * T * 128]),
        }
        for name, dat, Pdim in [
            ("s_n", s[lo:hi], DS), ("v_n", v[lo:hi], DV),
            ("sclsb_n", s_cls[batch_np[lo:hi]], DS),
            ("vclsb_n", v_cls[batch_np[lo:hi]], DV),
        ]:
            a = np.zeros((Pdim, N_pad), np.float32)
            a[:, :ncn] = dat.T
            m[name] = a
        goh = np.zeros((N_pad, 128), np.float32)
        goh[np.arange(ncn), bloc] = 1.0
        m["gh_oh"] = goh
        gohT = np.zeros((128, N_pad), np.float32)
        gohT[bloc, np.arange(ncn)] = 1.0
        m["gh_ohT"] = gohT
        ic = np.ones((128, 1), np.float32)
        ic[:G, 0] = inv_cnt[c * G:(c + 1) * G]
        m["invc"] = ic
        m["scls_g"] = np.ascontiguousarray(s_cls[c * G:(c + 1) * G].T)
        m["vcls_g"] = np.ascontiguousarray(v_cls[c * G:(c + 1) * G].T)
        m.update(weights)
        in_maps.append(m)

    meta = dict(T=T, S_list=tuple(int(s) for s in S_list), N_pad=N_pad, G=G,
                n_lo=n_lo, n_hi=n_hi, N=N, B=B)
    return meta, in_maps


def kernel(s, v, s_edge, v_edge, edge_index, s_cls, v_cls, batch, params):
    meta, in_maps = _prep(s, v, s_edge, v_edge, edge_index, s_cls, v_cls,
                          batch, params)
    key = (meta["T"], meta["S_list"], meta["N_pad"], meta["G"], USE_F32R)
    if key not in _BUILD_CACHE:
        _BUILD_CACHE[key] = _build_nc(meta["T"], meta["S_list"], meta["N_pad"],
                                      meta["G"])
    nc = _BUILD_CACHE[key]
    res = run_bass_kernel_spmd(nc, in_maps, list(range(C)))

    N, B, G = meta["N"], meta["B"], meta["G"]
    s_out = np.zeros((N, DS), np.float32)
    v_out = np.zeros((N, DV), np.float32)
    scls_out = np.zeros((B, DS), np.float32)
    vcls_out = np.zeros((B, DV), np.float32)
    for c in range(C):
        lo, hi = int(meta["n_lo"][c]), int(meta["n_hi"][c])
        ncn = hi - lo
        r = res.results[c]
        s_out[lo:hi] = r["s_out"][:, :ncn].T
        v_out[lo:hi] = r["v_out"][:, :ncn].T
        scls_out[c * G:(c + 1) * G] = r["scls_out"].T
        vcls_out[c * G:(c + 1) * G] = r["vcls_out"].T
    return s_out, v_out, scls_out, vcls_out


# revision 29
# speedup vs baseline: 1.0371x; 1.0001x over previous
"""Trainium2 Bass kernel for nn_AllInOneLayer (GNN message-passing layer).

Sharding: data-parallel over graphs. 8 cores, B/8 graphs each; nodes/edges
of a core's graphs are contiguous (batch is sorted). Edges are assigned to
the core that owns their DESTINATION node (row = edge_index[0]); within a
core they are grouped per 128-node tile with a uniform padded slot count S,
so the segment softmax and scatter-adds become core-local PE matmuls with
host-built one-hot matrices. All float math runs on the device; the host
only does index manipulation / gathers / layout packing.
"""

import contextlib
import math

import numpy as np
import ml_dtypes
_bf16 = ml_dtypes.bfloat16

import concourse.bass as bass
import concourse.tile as tile
from concourse import mybir
from concourse.bass_utils import run_bass_kernel_spmd
from concourse.masks import make_identity

AF = mybir.ActivationFunctionType
OP = mybir.AluOpType
F32 = mybir.dt.float32
F32R = mybir.dt.float32r
BF16 = mybir.dt.bfloat16

C = 8           # cores
DS, DV, DU, HH = 128, 15, 128, 4
HID_A = DS + DV           # 143, attn hidden per head
MA = HH * HID_A           # 572, stacked attn hidden
USE_F32R = True           # relaxed-precision matmuls (4x faster, ~1e-4 rel err)
FR = F32R if USE_F32R else F32   # dtype for every matmul-feeding tensor

_BUILD_CACHE = {}


def _r(ap):
    """Matmul operands already carry FR dtype; kept for call-site clarity."""
    return ap


def _split_waits(nc, limit=1):
    """This toolchain's walrus accepts at most one sync-wait per instruction;
    move excess waits onto preceding same-engine NoOps."""
    for f in nc.m.functions:
        for bb in f.blocks:
            new = []
            for inst in bb.instructions:
                si = inst.sync_info
                waits = list(si.on_wait) if si else []
                if len(waits) > limit:
                    chunks = [waits[i:i + limit] for i in range(0, len(waits), limit)]
                    for ch in chunks[:-1]:
                        nop = mybir.InstNoOp(
                            name=nc.get_next_instruction_name(), ins=[], outs=[])
                        nop.engine = inst.engine
                        nop.sync_info = mybir.SyncInfo(on_wait=list(ch), on_update=[])
                        new.append(nop)
                    inst.sync_info = mybir.SyncInfo(
                        on_wait=list(chunks[-1]), on_update=list(si.on_update))
                new.append(inst)
            bb.instructions = new


# weight/bias dram shapes (lhsT layout [K_in, M_out]; biases [p, cols])
_WSHAPES = [
    ("wn1", [HID_A, 2 * DU]), ("wn2", [2 * DU, DU]),
    ("we1", [HID_A, 2 * DU]), ("we2", [2 * DU, DU]),
    ("wm1", [3 * DU, 2 * DU]), ("wf", [6 * DU, 2 * DU]),
    ("wm2", [2 * DU, DS]),
    ("ws1", [DS, DS // 2]), ("ws2", [DS // 2, 1]),
    ("wa1", [3 * DS, MA]), ("wa1v", [80, MA]), ("wa2", [MA, HH]),
    ("wnev", [80, 2 * DU]),
    ("wsd1", [3 * DS, DS]), ("wsd2", [DS, DS]),
    ("wvd1", [3 * DV, DV]), ("wvd2", [DV, DV]),
    ("wg1", [DS + 3, DS]), ("wg2", [DS, 1]),
    ("wsc1", [3 * DS, DS]), ("wsc2", [DS, DS]),
    ("wvc1", [3 * DV, DV]), ("wvc2", [DV, DV]),
    ("bn1", [128, 2]), ("be1", [128, 2]),
    ("bm1", [128, 2]), ("bm2", [128, 1]),
    ("bs1", [64, 1]), ("bs2", [1, 1]),
    ("ba1", [128, 5]), ("ba2", [4, 1]),
    ("bsd1", [128, 1]), ("bsd2", [128, 1]),
    ("bvd1", [15, 1]), ("bvd2", [15, 1]),
    ("bg1", [128, 1]), ("bg2", [1, 1]),
    ("bsc1", [128, 1]), ("bsc2", [128, 1]),
    ("bvc1", [15, 1]), ("bvc2", [15, 1]),
    ("gnw", [128, 1]), ("gnb", [128, 1]), ("gnms", [128, 1]),
    ("lnw", [128, 1]), ("lnb", [128, 1]),
]
# loaded whole into one sbuf tile (partition dim <= 128)
_WSMALL = {"ws1", "ws2", "wsd2", "wvd2", "wg2", "wsc2",
           "wvc2", "bn1", "be1", "bm1", "bm2", "bs1", "bs2", "ba1", "ba2",
           "bsd1", "bsd2", "bvd1", "bvd2", "bg1", "bg2", "bsc1", "bsc2",
           "bvc1", "bvc2", "gnw", "gnb", "gnms", "lnw", "lnb"}



def _chunk_widths(S_t):
    """Split S_t (multiple of 128) into ceil(S_t/512) balanced widths,
    each a multiple of 128 and <= 512."""
    nch = math.ceil(S_t / 512)
    nsub = S_t // 128
    base = nsub // nch
    rem = nsub - base * nch
    return [(base + (1 if i < rem else 0)) * 128 for i in range(nch)]


def _build_nc(T, S_list, N_pad, G):
    assert G % 2 == 0, "graphs per core must be even (f32r even-N rule)"
    """Emit the per-core Bass program. T node tiles of 128, S_list[t] edge
    slots per node tile (mult of 128), N_pad = T*128 nodes, G graphs/core."""
    nc = bass.Bass()
    dp = nc.declare_dram_parameter
    S_list = list(S_list)
    S_off = [0]
    for s_ in S_list:
        S_off.append(S_off[-1] + s_)
    Stot = S_off[-1]
    S_max = max(S_list)
    KSUB = S_max // 128
    NCH = math.ceil(N_pad / 512)

    sss_e = dp("sss", [DS, 3 * Stot], FR, isOutput=False)  # si|sj|se per chunk
    vvv_e = dp("vvv", [80, Stot], FR, isOutput=False)      # vi@0, vj@32, ve@64
    oh_e = dp("oh", [Stot, 128], BF16, isOutput=False)     # [edge, node] one-hot
    ohT_e = dp("ohT", [T * 128, S_max], FR, isOutput=False)  # [node, edge] one-hot

    s_nd = dp("s_n", [DS, N_pad], FR, isOutput=False)
    v_nd = dp("v_n", [DV, N_pad], FR, isOutput=False)
    sclsb_nd = dp("sclsb_n", [DS, N_pad], FR, isOutput=False)
    vclsb_nd = dp("vclsb_n", [DV, N_pad], FR, isOutput=False)
    gh_oh = dp("gh_oh", [N_pad, 128], FR, isOutput=False)
    gh_ohT = dp("gh_ohT", [128, N_pad], FR, isOutput=False)
    invc_d = dp("invc", [128, 1], F32, isOutput=False)
    scls_d = dp("scls_g", [DS, G], FR, isOutput=False)
    vcls_d = dp("vcls_g", [DV, G], FR, isOutput=False)
    wd = {name: dp(name, shp, FR if name.startswith("w") else F32, isOutput=False)
          for name, shp in _WSHAPES}

    s_out_e = dp("s_out", [DS, N_pad], FR, isOutput=True)
    v_out_e = dp("v_out", [DV, N_pad], FR, isOutput=True)
    scls_out_e = dp("scls_out", [DS, G], F32, isOutput=True)
    vcls_out_e = dp("vcls_out", [DV, G], F32, isOutput=True)

    with nc.allow_low_precision(reason="float32r compute"), \
         tile.TileContext(nc) as tc, contextlib.ExitStack() as ctx:
        ep = ctx.enter_context  # shorthand
        cst = ep(tc.tile_pool(name="cst", bufs=1))
        per = ep(tc.tile_pool(name="per", bufs=1))
        ectx = contextlib.ExitStack()   # edge-stage pools, freed before node stage
        eep = ectx.enter_context
        epin = eep(tc.tile_pool(name="epin", bufs=3))
        epu = eep(tc.tile_pool(name="epu", bufs=5))
        eph = eep(tc.tile_pool(name="eph", bufs=3))
        uhp = eep(tc.tile_pool(name="uhp", bufs=9))
        epm = eep(tc.tile_pool(name="epm", bufs=3))
        epm1 = eep(tc.tile_pool(name="epm1", bufs=1))
        bufp = eep(tc.tile_pool(name="bufp", bufs=KSUB + 6))
        ohp = eep(tc.tile_pool(name="ohp", bufs=1))
        wmp = eep(tc.tile_pool(name="wmp", bufs=3))
        # PSUM: 8 banks total -> pb(3) + ptr(2) + pz(1) + pacc(2)
        pb = ep(tc.tile_pool(name="pb", bufs=4, space="PSUM"))
        ptr = ep(tc.tile_pool(name="ptr", bufs=2, space="PSUM"))
        pzp = ep(tc.tile_pool(name="pz", bufs=1, space="PSUM"))
        pacc = ep(tc.tile_pool(name="pacc", bufs=1, space="PSUM"))

        def PB(p_, f_):
            return pb.tile([p_, f_], F32, tag="pb", name="pbt")

        def PTR(p_, f_, dt_=F32):
            return ptr.tile([p_, f_], dt_, tag="ptr", name="ptrt")

        # ---- constants ----
        ident_f = cst.tile([128, 128], F32)
        make_identity(nc, ident_f[:])
        ident = cst.tile([128, 128], FR)
        nc.vector.tensor_copy(ident[:], ident_f[:])
        cscr = cst.tile([128, 1], F32)
        nc.vector.memset(cscr[:], 1.0)
        ones15 = cst.tile([15, 1], FR)
        nc.vector.tensor_copy(ones15[:], cscr[0:15, :])
        cscr1x = cst.tile([1, 128], F32)
        nc.vector.memset(cscr1x[:], 1.0)
        ones1x = cst.tile([1, 128], FR)
        nc.vector.tensor_copy(ones1x[:], cscr1x[:])
        cscr2 = cst.tile([128, 1], F32)
        nc.vector.memset(cscr2[:], 1.0 / DS)
        oo128 = cst.tile([128, 1], FR)
        nc.vector.tensor_copy(oo128[:], cscr2[:])
        eps5 = cst.tile([128, 1], F32)
        nc.vector.memset(eps5[:], 1e-5)

        W = {}
        for name in _WSMALL:
            W[name] = cst.tile(list(wd[name].shape), FR if name.startswith("w") else F32,
                               tag=f"w_{name}", name=f"w_{name}")
            nc.sync.dma_start(out=W[name][:], in_=wd[name][:])

        def ksplit(name, splits):
            """Load dram weight rows into separate tiles per K-chunk."""
            tiles, r0 = [], 0
            for kk in splits:
                t_ = cst.tile([kk, wd[name].shape[1]], FR, tag=f"{name}_{r0}", name=f"{name}_{r0}")
                nc.sync.dma_start(out=t_[:], in_=wd[name][r0:r0 + kk, :])
                tiles.append(t_)
                r0 += kk
            return tiles

        wn1_k = ksplit("wn1", [128])
        we1_k = ksplit("we1", [128])
        wa1v = cst.tile([80, MA], FR)
        nc.sync.dma_start(out=wa1v[:], in_=wd["wa1v"][:])
        wnev = cst.tile([80, 2 * DU], FR)
        nc.sync.dma_start(out=wnev[:], in_=wd["wnev"][:])
        wf_k = ksplit("wf", [128] * 6)
        wm2_k = ksplit("wm2", [128, 128])
        wa1_k = ksplit("wa1", [128, 128, 128])  # si,sj,se rows
        wa2_k = ksplit("wa2", [128, 128, 128, 128, 60])
        wsd1_k = ksplit("wsd1", [128, 128, 128])
        wsc1_k = ksplit("wsc1", [128, 128, 128])
        wvd1_k = ksplit("wvd1", [15, 15, 15])
        wvc1_k = ksplit("wvc1", [15, 15, 15])
        wg1_k = ksplit("wg1", [1, 1, 1, 128])

        invc = cst.tile([128, 1], F32)
        nc.sync.dma_start(out=invc[:], in_=invc_d[:])
        scls_s = cst.tile([DS, G], FR)
        nc.sync.dma_start(out=scls_s[:], in_=scls_d[:])
        vcls_s = cst.tile([DV, G], FR)
        nc.sync.dma_start(out=vcls_s[:], in_=vcls_d[:])

        # ---- persistent buffers ----
        agg_sb = [per.tile([128, 144], FR, tag=f"agg{t}", name=f"agg{t}") for t in range(T)]
        sagg_fn = per.tile([DS, N_pad], FR)
        vagg_fn = per.tile([DV, N_pad], FR)
        x_fn = per.tile([DS, N_pad], FR)       # x, later overwritten by xc
        sout_fn = per.tile([DS, N_pad], FR)
        vout_fn = per.tile([16, N_pad], FR)
        mean_gf = per.tile([128, 128], FR)
        nm_g = per.tile([128, 144], FR)
        em_g = per.tile([128, 144], FR)

        # ================= EDGE STAGE =================
        for t in range(T):
            S_t = S_list[t]
            KSUB_t = S_t // 128
            oh_t = ohp.tile([128, KSUB, 128], BF16, tag="oh", name="oh_t")
            nc.sync.dma_start(
                out=oh_t[:, 0:KSUB_t, :],
                in_=oh_e[S_off[t]:S_off[t + 1], :].rearrange(
                    "(k p) n -> p k n", p=128))
            ohT_t = ohp.tile([128, S_max], FR, tag="ohT", name="ohT_t")
            nc.sync.dma_start(out=ohT_t[:, 0:S_t],
                              in_=ohT_e[t * 128:(t + 1) * 128, 0:S_t])

            psz = pzp.tile([128, 4], F32, tag="pz")
            bufs_t = []
            widths = _chunk_widths(S_t)
            w_off = [0]
            for w_ in widths:
                w_off.append(w_off[-1] + w_)
            for ci, cw in enumerate(widths):
                e0 = S_off[t] + w_off[ci]
                sss_c = epin.tile([DS, 3 * cw], FR, tag="sss")
                nc.sync.dma_start(out=sss_c[:], in_=sss_e[:, 3 * e0:3 * e0 + 3 * cw])
                vvv_c = epin.tile([80, cw], FR, tag="vvv")
                nc.sync.dma_start(out=vvv_c[:], in_=vvv_e[:, e0:e0 + cw])
                si_c, sj_c, se_c = (sss_c[:, 0:cw], sss_c[:, cw:2 * cw],
                                    sss_c[:, 2 * cw:3 * cw])
                vi_c, vj_c, ve_c = (vvv_c[0:15, :], vvv_c[32:47, :],
                                    vvv_c[64:79, :])

                # interleaved emission: attn m-chunks woven between u-MLP
                # stages so PE always has independent matmuls to run while
                # ACT processes silus.
                def u_hid(s_in, v_in, v_w1, w1k, b1, m):
                    ph = PB(128, cw)
                    nc.tensor.matmul(ph[:], _r(w1k[0][:, 128 * m:128 * (m + 1)]),
                                     _r(s_in), start=True, stop=False)
                    nc.tensor.matmul(ph[:], _r(v_w1[:, 128 * m:128 * (m + 1)]),
                                     _r(v_in), start=False, stop=True)
                    h = uhp.tile([128, cw], FR, tag="uh", name="h")
                    nc.scalar.activation(h[:], ph[:], AF.Silu, bias=b1[:, m:m + 1])
                    return h

                praw = PB(4, cw)

                def attn_m(m):
                    mw = min(128, MA - 128 * m)
                    ph = PB(128, cw)
                    ins = [(wa1_k[0][:, 128 * m:128 * m + mw], si_c),
                           (wa1_k[1][:, 128 * m:128 * m + mw], sj_c),
                           (wa1_k[2][:, 128 * m:128 * m + mw], se_c),
                           (wa1v[:, 128 * m:128 * m + mw], vvv_c[:])]
                    for ki, (wt, xin) in enumerate(ins):
                        nc.tensor.matmul(ph[:mw, :], _r(wt), _r(xin),
                                         start=(ki == 0), stop=(ki == 3))
                    h = eph.tile([128, cw], FR, tag="ah", name="ah")
                    nc.scalar.activation(h[:mw, :], ph[:mw, :], AF.Silu,
                                         bias=W["ba1"][:mw, m:m + 1])
                    nc.tensor.matmul(praw[:], _r(wa2_k[m][:]), _r(h[:mw, :]),
                                     start=(m == 0), stop=(m == 4))

                attn_m(0)
                ui0 = u_hid(si_c, vi_c, wnev[0:15, :], wn1_k, W["bn1"], 0)
                ui1 = u_hid(si_c, vi_c, wnev[0:15, :], wn1_k, W["bn1"], 1)
                attn_m(1)
                uj0 = u_hid(sj_c, vj_c, wnev[32:47, :], wn1_k, W["bn1"], 0)
                uj1 = u_hid(sj_c, vj_c, wnev[32:47, :], wn1_k, W["bn1"], 1)
                attn_m(2)
                ue0 = u_hid(se_c, ve_c, wnev[64:79, :], we1_k, W["be1"], 0)
                ue1 = u_hid(se_c, ve_c, wnev[64:79, :], we1_k, W["be1"], 1)
                attn_m(3)
                uhids = [ui0, ui1, uj0, uj1, ue0, ue1]
                attn_m(4)

                # msg L1 folded over u-hiddens: Wf = W2_{node,edge} @ W1_msg chunks
                mh = []
                for m in range(2):
                    ph = PB(128, cw)
                    for ki, hh in enumerate(uhids):
                        nc.tensor.matmul(ph[:], _r(wf_k[ki][:, 128 * m:128 * (m + 1)]),
                                         _r(hh[:]), start=(ki == 0), stop=(ki == 5))
                    h = uhp.tile([128, cw], FR, tag="uh", name="mhh")
                    nc.scalar.activation(h[:], ph[:], AF.Silu, bias=W["bm1"][:, m:m + 1])
                    mh.append(h)
                pm = PB(128, cw)
                nc.tensor.matmul(pm[:], _r(wm2_k[0][:]), _r(mh[0][:]),
                                 start=True, stop=False)
                nc.tensor.matmul(pm[:], _r(wm2_k[1][:]), _r(mh[1][:]),
                                 start=False, stop=True)
                msg = epm.tile([128, cw], FR, tag="msg")
                nc.vector.tensor_scalar_add(msg[:], pm[:], W["bm2"][:, 0:1])

                # coeff = mlp2(msg) 128->64->1
                pch = PB(64, cw)
                nc.tensor.matmul(pch[:], _r(W["ws1"][:]), _r(msg[:]),
                                 start=True, stop=True)
                chh = eph.tile([64, cw], FR, tag="ch")
                nc.scalar.activation(chh[:], pch[:], AF.Silu, bias=W["bs1"][:, 0:1])
                pco = PB(1, cw)
                nc.tensor.matmul(pco[:], _r(W["ws2"][:]), _r(chh[:]),
                                 start=True, stop=True)
                coeff = epm1.tile([1, cw], FR, tag="cf")
                nc.vector.tensor_scalar_add(coeff[:], pco[:], W["bs2"][:, 0:1])

                # v_msg = v_edge * coeff (coeff broadcast to 15 partitions);
                # er lands in rows 32:36 of the same tile so one transpose
                # moves both to [edge_p, feat].
                pcb = PB(15, cw)
                nc.tensor.matmul(pcb[:], _r(ones1x[0:1, 0:15]), _r(coeff[:]),
                                 start=True, stop=True)
                vmsg_c = epm1.tile([36, cw], FR, tag="vmsgc")
                nc.vector.tensor_mul(vmsg_c[0:15, :], ve_c, pcb[:])
                nc.scalar.activation(vmsg_c[32:36, :], praw[:], AF.Exp,
                                     bias=W["ba2"][:, 0:1])

                # transpose msg/(vmsg|er) to [edge_p, feat]; z aggregation
                for kk in range(cw // 128):
                    kg = w_off[ci] // 128 + kk
                    sl = slice(128 * kk, 128 * (kk + 1))
                    buf = bufp.tile([128, 164], BF16, tag="buf")
                    bufs_t.append(buf)
                    ptm = PTR(128, 128, FR)
                    nc.tensor.transpose(ptm[:], msg[:, sl], ident[:])
                    nc.vector.tensor_copy(buf[:, 0:128], ptm[:])
                    ptv = PTR(128, 36, FR)
                    nc.tensor.transpose(ptv[:], vmsg_c[:, sl], ident[0:36, 0:36])
                    nc.vector.tensor_copy(buf[:, 128:164], ptv[:])
                    nc.tensor.matmul(psz[:], _r(oh_t[:, kg, :]), _r(buf[:, 160:164]),
                                     start=(kg == 0), stop=(kg == KSUB_t - 1))

            # z -> 1/max(z, eps)
            rz = epm.tile([128, 4], FR, tag="rz")
            nc.vector.tensor_scalar(out=rz[:], in0=psz[:], scalar1=1e-30,
                                    scalar2=4.0, op0=OP.max, op1=OP.mult)
            nc.vector.reciprocal(rz[:], rz[:])

            # pass B: broadcast 1/z to edges, weight msgs, aggregate
            pag = pacc.tile([128, 144], F32, tag="pacc")
            for kg in range(KSUB_t):
                buf = bufs_t[kg]
                pzb = PTR(128, 4)
                nc.tensor.matmul(pzb[:], _r(ohT_t[:, 128 * kg:128 * (kg + 1)]),
                                 _r(rz[:]), start=True, stop=True)
                scr = wmp.tile([128, 4], F32, tag="scr")
                wT = wmp.tile([128, 1], F32, tag="wT")
                nc.vector.tensor_mul(scr[:], buf[:, 160:164], pzb[:])
                nc.vector.reduce_sum(wT[:], scr[:], axis=mybir.AxisListType.X)
                wmsg = wmp.tile([128, 144], BF16, tag="wmsg")
                nc.vector.tensor_scalar_mul(wmsg[:], buf[:, 0:144], wT[:, 0:1])
                nc.tensor.matmul(pag[:], _r(oh_t[:, kg, :]), _r(wmsg[:]),
                                 start=(kg == 0), stop=(kg == KSUB_t - 1))
            nc.vector.tensor_copy(agg_sb[t][:], pag[:])
            pta = PTR(128, 128, FR)
            nc.tensor.transpose(pta[:], agg_sb[t][:, 0:128], ident[:])
            nc.vector.tensor_copy(sagg_fn[:, 128 * t:128 * (t + 1)], pta[:])
            ptb = PTR(16, 128, FR)
            nc.tensor.transpose(ptb[:], agg_sb[t][:, 128:144], ident[:])
            nc.vector.tensor_copy(vagg_fn[:, 128 * t:128 * (t + 1)], ptb[0:15, :])

        # ================= NODE STAGE =================
        ectx.close()
        nsp = ep(tc.tile_pool(name="nsp", bufs=2))
        nsp1 = ep(tc.tile_pool(name="nsp1", bufs=1))
        gp = ep(tc.tile_pool(name="gp", bufs=1))
        ghoh = gp.tile([128, T * 128], FR)
        for t in range(T):
            nc.sync.dma_start(out=ghoh[:, 128 * t:128 * (t + 1)],
                              in_=gh_oh[128 * t:128 * (t + 1), :])
        ghohT = gp.tile([128, N_pad], FR)
        nc.sync.dma_start(out=ghohT[:], in_=gh_ohT[:])
        for ci in range(NCH):
            n0 = ci * 512
            cw = min(512, N_pad - n0)
            sl = slice(n0, n0 + cw)
            s_c = nsp.tile([DS, cw], FR, tag="s_c")
            nc.sync.dma_start(out=s_c[:], in_=s_nd[:, sl])
            scb_c = nsp.tile([DS, cw], FR, tag="scb")
            nc.sync.dma_start(out=scb_c[:], in_=sclsb_nd[:, sl])
            v_c = nsp1.tile([DV, cw], FR, tag="v_c")
            nc.sync.dma_start(out=v_c[:], in_=v_nd[:, sl])
            vcb_c = nsp1.tile([DV, cw], FR, tag="vcb")
            nc.sync.dma_start(out=vcb_c[:], in_=vclsb_nd[:, sl])

            # s_delta MLP 384->128->128 ; x = s + s_delta
            ph = PB(128, cw)
            for ki, xin in enumerate([s_c[:], sagg_fn[:, sl], scb_c[:]]):
                nc.tensor.matmul(ph[:], _r(wsd1_k[ki][:]), _r(xin),
                                 start=(ki == 0), stop=(ki == 2))
            h = nsp.tile([128, cw], FR, tag="nh")
            nc.scalar.activation(h[:], ph[:], AF.Silu, bias=W["bsd1"][:, 0:1])
            po = PB(128, cw)
            nc.tensor.matmul(po[:], _r(W["wsd2"][:]), _r(h[:]), start=True, stop=True)
            nc.vector.scalar_tensor_tensor(
                out=x_fn[:, sl], in0=po[:], scalar=W["bsd2"][:, 0:1],
                in1=s_c[:], op0=OP.add, op1=OP.add)

            # v_delta MLP 45->15->15
            pvh = PB(15, cw)
            for ki, xin in enumerate([v_c[:], vagg_fn[:, sl], vcb_c[:]]):
                nc.tensor.matmul(pvh[:], _r(wvd1_k[ki][:]), _r(xin),
                                 start=(ki == 0), stop=(ki == 2))
            vh = nsp1.tile([15, cw], FR, tag="vh")
            nc.scalar.activation(vh[:], pvh[:], AF.Silu, bias=W["bvd1"][:, 0:1])
            pvo = PB(15, cw)
            nc.tensor.matmul(pvo[:], _r(W["wvd2"][:]), _r(vh[:]), start=True, stop=True)
            vd_c = nsp1.tile([DV, cw], FR, tag="vd_c")
            nc.vector.tensor_scalar_add(vd_c[:], pvo[:], W["bvd2"][:, 0:1])

            # gate: concat(|v|, |vd|, cos, s) -> 128 -> 1 -> sigmoid
            t2 = nsp1.tile([15, cw], FR, tag="t2")
            nc.vector.tensor_mul(t2[:], v_c[:], v_c[:])
            pvn = PB(1, cw)
            nc.tensor.matmul(pvn[:], _r(ones15[:]), _r(t2[:]), start=True, stop=True)
            vn = nsp1.tile([1, cw], FR, tag="vn")
            nc.scalar.activation(vn[:], pvn[:], AF.Sqrt)
            t3 = nsp1.tile([15, cw], FR, tag="t3")
            nc.vector.tensor_mul(t3[:], vd_c[:], vd_c[:])
            pdn = PB(1, cw)
            nc.tensor.matmul(pdn[:], _r(ones15[:]), _r(t3[:]), start=True, stop=True)
            dn = nsp1.tile([1, cw], FR, tag="dn")
            nc.scalar.activation(dn[:], pdn[:], AF.Sqrt)
            t4 = nsp1.tile([15, cw], FR, tag="t4")
            nc.vector.tensor_mul(t4[:], v_c[:], vd_c[:])
            pdo = PB(1, cw)
            nc.tensor.matmul(pdo[:], _r(ones15[:]), _r(t4[:]), start=True, stop=True)
            den = nsp1.tile([1, cw], F32, tag="den")
            nc.vector.tensor_mul(den[:], vn[:], dn[:])
            nc.vector.tensor_scalar_add(den[:], den[:], 1e-6)
            nc.vector.reciprocal(den[:], den[:])
            cosn = nsp1.tile([1, cw], FR, tag="cosn")
            nc.vector.tensor_mul(cosn[:], pdo[:], den[:])
            pgh = PB(128, cw)
            nc.tensor.matmul(pgh[:], _r(wg1_k[0][:]), _r(vn[:]), start=True, stop=False)
            nc.tensor.matmul(pgh[:], _r(wg1_k[1][:]), _r(dn[:]), start=False, stop=False)
            nc.tensor.matmul(pgh[:], _r(wg1_k[2][:]), _r(cosn[:]), start=False, stop=False)
            nc.tensor.matmul(pgh[:], _r(wg1_k[3][:]), _r(s_c[:]),
                             start=False, stop=True)
            gh = nsp.tile([128, cw], FR, tag="nh")
            nc.scalar.activation(gh[:], pgh[:], AF.Silu, bias=W["bg1"][:, 0:1])
            pgo = PB(1, cw)
            nc.tensor.matmul(pgo[:], _r(W["wg2"][:]), _r(gh[:]), start=True, stop=True)
            gate = nsp1.tile([1, cw], FR, tag="gate")
            nc.scalar.activation(gate[:], pgo[:], AF.Sigmoid, bias=W["bg2"][:, 0:1])
            pgb = PB(15, cw)
            nc.tensor.matmul(pgb[:], _r(ones1x[0:1, 0:15]), _r(gate[:]),
                             start=True, stop=True)
            nc.vector.tensor_mul(vout_fn[0:15, sl], vd_c[:], pgb[:])
            nc.vector.tensor_add(vout_fn[0:15, sl], vout_fn[0:15, sl], v_c[:])

        # ---- GraphNorm (one-pass stats: aggregate [x | x^2] together) ----
        pgs = pacc.tile([128, 256], F32, tag="pacc", name="pgs")
        for t in range(T):
            sq = nsp.tile([128, 128], FR, tag="sq")
            nc.vector.tensor_mul(sq[:], x_fn[:, 128 * t:128 * (t + 1)],
                                 x_fn[:, 128 * t:128 * (t + 1)])
            ptx = PTR(128, 128, FR)
            nc.tensor.transpose(ptx[:], x_fn[:, 128 * t:128 * (t + 1)], ident[:])
            ptq = PTR(128, 128, FR)
            nc.tensor.transpose(ptq[:], sq[:], ident[:])
            xts = nsp.tile([128, 256], FR, tag="xts")
            nc.vector.tensor_copy(xts[:, 0:128], ptx[:])
            nc.vector.tensor_copy(xts[:, 128:256], ptq[:])
            nc.tensor.matmul(pgs[:], _r(ghoh[:, 128 * t:128 * (t + 1)]), _r(xts[:]),
                             start=(t == 0), stop=(t == T - 1))
        nc.vector.tensor_scalar_mul(mean_gf[:], pgs[:, 0:128], invc[:, 0:1])
        ex2_gf = per.tile([128, 128], FR)
        nc.vector.tensor_scalar_mul(ex2_gf[:], pgs[:, 128:256], invc[:, 0:1])
        for ci in range(NCH):
            n0 = ci * 512
            cw = min(512, N_pad - n0)
            sl = slice(n0, n0 + cw)
            pmb = PB(128, cw)
            nc.tensor.matmul(pmb[:], _r(mean_gf[:]), _r(ghohT[:, sl]),
                             start=True, stop=True)
            pxb = PB(128, cw)
            nc.tensor.matmul(pxb[:], _r(ex2_gf[:]), _r(ghohT[:, sl]),
                             start=True, stop=True)
            tmb = nsp.tile([128, cw], F32, tag="nh")      # gms * mean_b
            nc.vector.tensor_scalar_mul(tmb[:], pmb[:], W["gnms"][:, 0:1])
            qb = nsp.tile([128, cw], F32, tag="qb")       # 2*mean_b - tmb
            nc.vector.scalar_tensor_tensor(out=qb[:], in0=pmb[:], scalar=2.0,
                                           in1=tmb[:], op0=OP.mult, op1=OP.subtract)
            vb = nsp.tile([128, cw], F32, tag="vb")       # (2g-g^2) mean^2
            nc.vector.tensor_mul(vb[:], tmb[:], qb[:])
            nc.vector.tensor_sub(x_fn[:, sl], x_fn[:, sl], tmb[:])   # xc in place
            varb = nsp.tile([128, cw], F32, tag="vrb")
            nc.vector.tensor_tensor(out=varb[:], in0=pxb[:], in1=vb[:],
                                    op=OP.subtract)
            nc.scalar.activation(varb[:], varb[:], AF.Sqrt, bias=eps5[:, 0:1])
            nc.vector.reciprocal(varb[:], varb[:])
            nc.vector.tensor_mul(sout_fn[:, sl], x_fn[:, sl], varb[:])
            nc.vector.tensor_scalar(out=sout_fn[:, sl], in0=sout_fn[:, sl],
                                    scalar1=W["gnw"][:, 0:1], scalar2=W["gnb"][:, 0:1],
                                    op0=OP.mult, op1=OP.add)
            nc.sync.dma_start(out=s_out_e[:, sl], in_=sout_fn[:, sl])
            nc.sync.dma_start(out=v_out_e[:, sl], in_=vout_fn[0:15, sl])

        # ---- CLS stage ----
        pnm = pacc.tile([128, 144], F32, tag="pacc")
        pem = pacc.tile([128, 144], F32, tag="pacc")
        for t in range(T):
            so_t = nsp.tile([128, 144], FR, tag="sot")
            ptx = PTR(128, 128, FR)
            nc.tensor.transpose(ptx[:], sout_fn[:, 128 * t:128 * (t + 1)], ident[:])
            nc.vector.tensor_copy(so_t[:, 0:128], ptx[:])
            ptv = PTR(128, 16, FR)
            nc.tensor.transpose(ptv[:], vout_fn[:, 128 * t:128 * (t + 1)],
                                ident[0:16, 0:16])
            nc.vector.tensor_copy(so_t[:, 128:144], ptv[:])
            nc.tensor.matmul(pnm[:], _r(ghoh[:, 128 * t:128 * (t + 1)]), _r(so_t[:]),
                             start=(t == 0), stop=(t == T - 1))
            nc.tensor.matmul(pem[:], _r(ghoh[:, 128 * t:128 * (t + 1)]),
                             _r(agg_sb[t][:]), start=(t == 0), stop=(t == T - 1))
        nc.vector.tensor_scalar_mul(nm_g[:], pnm[:], invc[:, 0:1])
        nc.vector.tensor_scalar_mul(em_g[:], pem[:], invc[:, 0:1])

        def tr_fg(src_ap, P, Fw, tag):
            pt = PTR(Fw, P, FR)
            nc.tensor.transpose(pt[:], src_ap, ident[0:P, 0:P])
            dst = nsp.tile([Fw, P], FR, tag=tag)
            nc.vector.tensor_copy(dst[:], pt[:])
            return dst

        snm_fg = tr_fg(nm_g[0:G, 0:128], G, 128, "snm")
        sem_fg = tr_fg(em_g[0:G, 0:128], G, 128, "sem")
        vnm_fg = tr_fg(nm_g[0:G, 128:144], G, 16, "vnm")
        vem_fg = tr_fg(em_g[0:G, 128:144], G, 16, "vem")

        # s_cls update + layernorm over features
        ph = PB(128, G)
        for ki, xin in enumerate([scls_s[:], snm_fg[:], sem_fg[:]]):
            nc.tensor.matmul(ph[:], _r(wsc1_k[ki][:]), _r(xin),
                             start=(ki == 0), stop=(ki == 2))
        h = nsp.tile([128, G], FR, tag="nh")
        nc.scalar.activation(h[:], ph[:], AF.Silu, bias=W["bsc1"][:, 0:1])
        po = PB(128, G)
        nc.tensor.matmul(po[:], _r(W["wsc2"][:]), _r(h[:]), start=True, stop=True)
        yc = nsp.tile([128, G], FR, tag="yc")
        nc.vector.scalar_tensor_tensor(out=yc[:], in0=po[:], scalar=W["bsc2"][:, 0:1],
                                       in1=scls_s[:], op0=OP.add, op1=OP.add)
        pmu = PB(1, G)
        nc.tensor.matmul(pmu[:], _r(oo128[:]), _r(yc[:]), start=True, stop=True)
        mu = nsp.tile([1, G], FR, tag="mu")
        nc.vector.tensor_copy(mu[:], pmu[:])
        pmb = PB(128, G)
        nc.tensor.matmul(pmb[:], _r(ones1x[0:1, :]), _r(mu[:]), start=True, stop=True)
        ycc = nsp.tile([128, G], FR, tag="ycc")
        nc.vector.tensor_sub(ycc[:], yc[:], pmb[:])
        sqg = nsp.tile([128, G], FR, tag="sqg")
        nc.vector.tensor_mul(sqg[:], ycc[:], ycc[:])
        pvv = PB(1, G)
        nc.tensor.matmul(pvv[:], _r(oo128[:]), _r(sqg[:]), start=True, stop=True)
        rs = nsp.tile([1, G], FR, tag="rs")
        nc.scalar.activation(rs[:], pvv[:], AF.Sqrt, bias=eps5[0:1, 0:1])
        nc.vector.reciprocal(rs[:], rs[:])
        prb = PB(128, G)
        nc.tensor.matmul(prb[:], _r(ones1x[0:1, :]), _r(rs[:]), start=True, stop=True)
        sco = nsp.tile([128, G], F32, tag="sco")
        nc.vector.tensor_mul(sco[:], ycc[:], prb[:])
        nc.vector.tensor_scalar(out=sco[:], in0=sco[:], scalar1=W["lnw"][:, 0:1],
                                scalar2=W["lnb"][:, 0:1], op0=OP.mult, op1=OP.add)
        nc.sync.dma_start(out=scls_out_e[:], in_=sco[:])

        # v_cls update
        pvh = PB(15, G)
        for ki, xin in enumerate([vcls_s[:], vnm_fg[0:15, :], vem_fg[0:15, :]]):
            nc.tensor.matmul(pvh[:], _r(wvc1_k[ki][:]), _r(xin),
                             start=(ki == 0), stop=(ki == 2))
        vh = nsp.tile([15, G], FR, tag="vh")
        nc.scalar.activation(vh[:], pvh[:], AF.Silu, bias=W["bvc1"][:, 0:1])
        pvo = PB(15, G)
        nc.tensor.matmul(pvo[:], _r(W["wvc2"][:]), _r(vh[:]), start=True, stop=True)
        vco = nsp.tile([15, G], F32, tag="vco")
        nc.vector.scalar_tensor_tensor(out=vco[:], in0=pvo[:], scalar=W["bvc2"][:, 0:1],
                                       in1=vcls_s[:], op0=OP.add, op1=OP.add)
        nc.sync.dma_start(out=vcls_out_e[:], in_=vco[:])

    _split_waits(nc)
    return nc


def _prep(s, v, s_edge, v_edge, edge_index, s_cls, v_cls, batch, params):
    """Host-side index prep + sharding. Returns (meta, in_maps)."""
    s = np.asarray(s, np.float32)
    v = np.asarray(v, np.float32)
    s_edge = np.asarray(s_edge, np.float32)
    v_edge = np.asarray(v_edge, np.float32)
    s_cls = np.asarray(s_cls, np.float32)
    v_cls = np.asarray(v_cls, np.float32)
    ei = np.asarray(edge_index).astype(np.int64)
    batch_np = np.asarray(batch).astype(np.int64)
    row, col = ei[0], ei[1]
    N, E, B = s.shape[0], row.shape[0], s_cls.shape[0]
    G = B // C

    g_lo = np.arange(C, dtype=np.int64) * G
    n_lo = np.searchsorted(batch_np, g_lo).astype(np.int64)
    n_hi = np.append(n_lo[1:], N).astype(np.int64)
    n_cnt = n_hi - n_lo
    N_pad = 128 * math.ceil(n_cnt.max() / 128)
    T = N_pad // 128

    edge_core = np.searchsorted(n_hi, row, side="right")
    local_row = row - n_lo[edge_core]
    tile_of_edge = local_row // 128
    key = edge_core * T + tile_of_edge
    counts = np.bincount(key, minlength=C * T).reshape(C, T)
    S_list = np.maximum(512, 128 * np.ceil(counts.max(0) / 128).astype(np.int64))
    S_off = np.zeros(T + 1, np.int64)
    S_off[1:] = np.cumsum(S_list)
    Stot1 = int(S_off[-1])
    S_max = int(S_list.max())
    order = np.argsort(key, kind="stable")
    key_o = key[order]
    starts = np.zeros(C * T + 1, np.int64)
    starts[1:] = np.cumsum(counts.reshape(-1))
    rank = np.arange(E, dtype=np.int64) - starts[key_o]
    core_o = key_o // T
    tile_o = key_o % T
    gslot = core_o * Stot1 + S_off[tile_o] + rank
    row_o, col_o = row[order], col[order]
    lr_o = local_row[order]

    CTS = C * Stot1
    si_all = np.zeros((CTS, DS), np.float32)
    sj_all = np.zeros((CTS, DS), np.float32)
    se_all = np.zeros((CTS, DS), np.float32)
    vi_all = np.zeros((CTS, DV), np.float32)
    vj_all = np.zeros((CTS, DV), np.float32)
    ve_all = np.zeros((CTS, DV), np.float32)
    chunk_bounds = []
    for t in range(T):
        St = int(S_list[t])
        off = int(S_off[t])
        for w_ in _chunk_widths(St):
            chunk_bounds.append((off, off + w_))
            off += w_
    si_all[gslot] = s[row_o]
    sj_all[gslot] = s[col_o]
    se_all[gslot] = s_edge[order]
    vi_all[gslot] = v[row_o]
    vj_all[gslot] = v[col_o]
    ve_all[gslot] = v_edge[order]
    oh_all = np.zeros((CTS, 128), np.float32)
    oh_all[gslot, lr_o % 128] = 1.0
    ohT_all = np.zeros((C * T * 128, S_max), np.float32)
    ohT_all[key_o * 128 + lr_o % 128, rank] = 1.0

    cnt = np.bincount(batch_np, minlength=B).astype(np.float32)
    inv_cnt = 1.0 / np.maximum(cnt, 1.0)

    p = params
    wn1, bn1, wn2, bn2 = [np.asarray(a, np.float32) for a in p["node"]]
    we1, be1, we2, be2 = [np.asarray(a, np.float32) for a in p["edge"]]
    wm1, bm1, wm2, bm2 = [np.asarray(a, np.float32) for a in p["msg"]]
    ws1, bs1, ws2, bs2 = [np.asarray(a, np.float32) for a in p["s2v"]]
    wsd1, bsd1, wsd2, bsd2 = [np.asarray(a, np.float32) for a in p["s_delta"]]
    wvd1, bvd1, wvd2, bvd2 = [np.asarray(a, np.float32) for a in p["v_delta"]]
    wg1, bg1, wg2, bg2 = [np.asarray(a, np.float32) for a in p["gate"]]
    wsc1, bsc1, wsc2, bsc2 = [np.asarray(a, np.float32) for a in p["s_cls"]]
    wvc1, bvc1, wvc2, bvc2 = [np.asarray(a, np.float32) for a in p["v_cls"]]
    aw1 = np.asarray(p["attn_W1"], np.float32)
    ab1 = np.asarray(p["attn_b1"], np.float32)
    aw2 = np.asarray(p["attn_W2"], np.float32)
    ab2 = np.asarray(p["attn_b2"], np.float32)
    Fdim = aw1.shape[1]
    wa1f = np.ascontiguousarray(np.transpose(aw1, (1, 0, 2)).reshape(Fdim, MA))
    wa1 = wa1f[0:384]
    wa1v = np.zeros((80, MA), np.float32)
    wa1v[0:15] = wa1f[384:399]
    wa1v[32:47] = wa1f[399:414]
    wa1v[64:79] = wa1f[414:429]
    wnev = np.zeros((80, 2 * DU), np.float32)
    wnev[0:15] = wn1[128:143]
    wnev[32:47] = wn1[128:143]
    wnev[64:79] = we1[128:143]
    wa2 = np.zeros((MA, HH), np.float32)
    for hh in range(HH):
        wa2[hh * HID_A:(hh + 1) * HID_A, hh] = aw2[hh]
    ba1 = ab1.reshape(MA)
    ba1_pad = np.zeros((128, 5), np.float32)
    for m in range(5):
        mw = min(128, MA - 128 * m)
        ba1_pad[:mw, m] = ba1[128 * m:128 * m + mw]
    bm1_eff = bm1 + bn2 @ wm1[0:DU] + bn2 @ wm1[DU:2 * DU] + be2 @ wm1[2 * DU:3 * DU]
    wf = np.concatenate([wn2 @ wm1[0:DU], wn2 @ wm1[DU:2 * DU],
                         we2 @ wm1[2 * DU:3 * DU]], axis=0)  # [768, 256]

    weights = {
        "wn1": wn1, "wn2": wn2, "we1": we1, "we2": we2,
        "wm1": wm1, "wf": wf, "wm2": wm2, "ws1": ws1, "ws2": ws2,
        "wa1": wa1, "wa1v": wa1v, "wnev": wnev, "wa2": wa2,
        "wsd1": wsd1, "wsd2": wsd2,
        "wvd1": wvd1, "wvd2": wvd2, "wg1": wg1, "wg2": wg2,
        "wsc1": wsc1, "wsc2": wsc2, "wvc1": wvc1, "wvc2": wvc2,
        "bn1": np.ascontiguousarray(np.stack([bn1[:128], bn1[128:]], 1)),
        "be1": np.ascontiguousarray(np.stack([be1[:128], be1[128:]], 1)),
        "bm1": np.ascontiguousarray(np.stack([bm1_eff[:128], bm1_eff[128:]], 1)),
        "bm2": bm2[:, None], "bs1": bs1[:, None], "bs2": bs2[:, None],
        "ba1": ba1_pad, "ba2": ab2[:, None],
        "bsd1": bsd1[:, None], "bsd2": bsd2[:, None],
        "bvd1": bvd1[:, None], "bvd2": bvd2[:, None],
        "bg1": bg1[:, None], "bg2": bg2[:, None],
        "bsc1": bsc1[:, None], "bsc2": bsc2[:, None],
        "bvc1": bvc1[:, None], "bvc2": bvc2[:, None],
        "gnw": np.asarray(p["gn_weight"], np.float32)[:, None],
        "gnb": np.asarray(p["gn_bias"], np.float32)[:, None],
        "gnms": np.asarray(p["gn_mean_scale"], np.float32)[:, None],
        "lnw": np.asarray(p["ln_w"], np.float32)[:, None],
        "lnb": np.asarray(p["ln_b"], np.float32)[:, None],
    }
    weights = {k: np.ascontiguousarray(a) for k, a in weights.items()}

    in_maps = []
    for c in range(C):
        lo, hi, ncn = int(n_lo[c]), int(n_hi[c]), int(n_cnt[c])
        esl = slice(c * Stot1, (c + 1) * Stot1)
        bloc = batch_np[lo:hi] - c * G
        siT = si_all[esl].T
        sjT = sj_all[esl].T
        seT = se_all[esl].T
        cols = []
        for (a, b) in chunk_bounds:
            cols += [siT[:, a:b], sjT[:, a:b], seT[:, a:b]]
        sss = np.ascontiguousarray(np.concatenate(cols, axis=1))
        vvv = np.zeros((80, Stot1), np.float32)
        vvv[0:15] = vi_all[esl].T
        vvv[32:47] = vj_all[esl].T
        vvv[64:79] = ve_all[esl].T
        m = {
            "sss": sss,
            "vvv": vvv,
            "oh": np.ascontiguousarray(oh_all[esl]).astype(_bf16),
            "ohT": np.ascontiguousarray(ohT_all[c * T * 128:(c + 1) * T * 128]),
        }
        for name, dat, Pdim in [
            ("s_n", s[lo:hi], DS), ("v_n", v[lo:hi], DV),
            ("sclsb_n", s_cls[batch_np[lo:hi]], DS),
            ("vclsb_n", v_cls[batch_np[lo:hi]], DV),
        ]:
            a = np.zeros((Pdim, N_pad), np.float32)
            a[:, :ncn] = dat.T
            m[name] = a
        goh = np.zeros((N_pad, 128), np.float32)
        goh[np.arange(ncn), bloc] = 1.0
        m["gh_oh"] = goh
        gohT = np.zeros((128, N_pad), np.float32)
        gohT[bloc, np.arange(ncn)] = 1.0
        m["gh_ohT"] = gohT
        ic = np.ones((128, 1), np.float32)
        ic[:G, 0] = inv_cnt[c * G:(c + 1) * G]
        m["invc"] = ic
        m["scls_g"] = np.ascontiguousarray(s_cls[c * G:(c + 1) * G].T)
        m["vcls_g"] = np.ascontiguousarray(v_cls[c * G:(c + 1) * G].T)
        m.update(weights)
        in_maps.append(m)

    meta = dict(T=T, S_list=tuple(int(s) for s in S_list), N_pad=N_pad, G=G,
                n_lo=n_lo, n_hi=n_hi, N=N, B=B)
    return meta, in_maps


def kernel(s, v, s_edge, v_edge, edge_index, s_cls, v_cls, batch, params):
    meta, in_maps = _prep(s, v, s_edge, v_edge, edge_index, s_cls, v_cls,
                          batch, params)
    key = (meta["T"], meta["S_list"], meta["N_pad"], meta["G"], USE_F32R)
    if key not in _BUILD_CACHE:
        _BUILD_CACHE[key] = _build_nc(meta["T"], meta["S_list"], meta["N_pad"],
                                      meta["G"])
    nc = _BUILD_CACHE[key]
    try:
        res = run_bass_kernel_spmd(nc, in_maps, list(range(C)))
    except Exception:
        # one retry: the axon tunnel occasionally reports a transient
        # NRT_EXEC_UNIT_UNRECOVERABLE on a busy device
        import time as _time
        _time.sleep(5)
        res = run_bass_kernel_spmd(nc, in_maps, list(range(C)))

    N, B, G = meta["N"], meta["B"], meta["G"]
    s_out = np.zeros((N, DS), np.float32)
    v_out = np.zeros((N, DV), np.float32)
    scls_out = np.zeros((B, DS), np.float32)
    vcls_out = np.zeros((B, DV), np.float32)
    for c in range(C):
        lo, hi = int(meta["n_lo"][c]), int(meta["n_hi"][c])
        ncn = hi - lo
        r = res.results[c]
        s_out[lo:hi] = r["s_out"][:, :ncn].T
        v_out[lo:hi] = r["v_out"][:, :ncn].T
        scls_out[c * G:(c + 1) * G] = r["scls_out"].T
        vcls_out[c * G:(c + 1) * G] = r["vcls_out"].T
    return s_out, v_out, scls_out, vcls_out
